# revision 1
# baseline (speedup 1.0000x reference)
"""Bi-Mamba Trainium2 kernel (v4 — engine-rebalanced, LN folded into matmul).

Contract: kernel(**inputs) takes the FULL unsharded inputs (numpy) keyed as
reference.setup_inputs() and returns the FULL (8, 2048, 384) float32 output.

Sharding: pure data-parallel over batch — 8 samples, 8 cores, one sample per
core, no collectives. All weights are replicated per core.

Design notes (per core, channel-major [feature, token] layout):
  - LayerNorm is folded into the in_proj matmul: stats via PE ones-matmuls,
    rsqrt via bit-trick+Newton on token-major-reshaped 16-wide tiles, then
    the matmul input is x*rs and two extra contraction rows [mu*rs; 1] with
    host-prepared weights [-S[f]; c[f]] complete the normalized projection.
  - silu(x) is computed as x*(tanh(x/2)+1) = 2*silu(x) with the 0.5 folded
    into host-prepped weights (x_proj rows, Dp, out_proj): ACT tanh + one
    4x-mode tensor_scalar + one 2x-mode bf16 mul, no 1x-mode STT ops.
  - causal depthwise conv (k=4) runs on the PE as 4 diagonal-matrix matmuls
    accumulated in PSUM (diagonals built on the host).
  - everything that streams is bf16 (DVE TensorTensor is 2x only for 2-byte
    packed dtypes; matmuls are full rate in bf16).
  - scan: per (128-channel group, state): a = exp(A*dt) on ACT, b = dtx*B_rep
    (DVE/Pool), h = tensor_tensor_scan(a,b) [DVE-only, 1x], hc = h*C_rep
    (DVE/Pool), and y += hc happens on the PE as an identity-matmul PSUM
    accumulation (GH=2 groups in flight, 2x4 PSUM banks).
  - out_proj accumulates over the 6 channel groups in PSUM, then residual-add
    and a reversed store so the second block runs identical code on the
    flipped sequence.
  - two shared block-scope PSUM pools and a single scan scope avoid
    pool-release barriers inside the scan; the first broadcast DMAs are
    prefetched before the phase-A pool release barrier.
"""
import numpy as np
from contextlib import ExitStack

import concourse.bass as bass
import concourse.tile as tile
from concourse import bacc, mybir
from concourse.bass_utils import run_bass_kernel_spmd

F32 = mybir.dt.float32
BF16 = mybir.dt.bfloat16
AF = mybir.ActivationFunctionType
OP = mybir.AluOpType

D_MODEL = 384
D_INNER = 768
D_STATE = 16
DT_RANK = 24
K_CONV = 4
L = 2048
BATCH = 8
EPS = 1e-5
NG = D_INNER // 128      # 6 channel blocks of d_inner
NM = D_MODEL // 128      # 3 channel blocks of d_model


def _bcast_row(ap):
    """View a [1, L] AP as [128, L] with partition step 0 (replicated read)."""
    return bass.AP(tensor=ap.tensor, offset=ap.offset, ap=[[0, 128]] + list(ap.ap[1:]))


def build_module(debug=False, repeat=1, cfg=None):
    cfg = cfg or {}
    b_dve = cfg.get('b_dve', 1)        # of 4 slots, how many b-muls go to DVE
    hc_dve = cfg.get('hc_dve', 1)
    gate_eng = cfg.get('gate_eng', 'gpsimd')
    resid_eng = cfg.get('resid_eng', 'vector')
    GH = 2
    sc_bufs = cfg.get('sc_bufs', 2)
    PREF = cfg.get('pref', 2)          # broadcast loads prefetched pre-barrier

    nc = bacc.Bacc("TRN2", target_bir_lowering=False, debug=False)

    def din(name, shape, dt=F32):
        return nc.dram_tensor(name, shape, dt, kind="ExternalInput").ap()

    xT = din("xT", [D_MODEL, L])
    xTb = din("xTb", [D_MODEL, L], BF16)
    eye = din("eye", [128, 128], BF16)
    w_in = din("w_in", [2, D_MODEL, 2 * D_INNER], BF16)   # host: folded x norm_w
    w4 = din("w4", [2, 2, 2 * D_INNER], BF16)             # [-S[f]; c[f]] rows
    w_out = din("w_out", [2, D_INNER, D_MODEL], BF16)     # host-scaled x0.5
    w_xp = din("w_xp", [2, D_INNER, 56], BF16)            # host-scaled (see prep)
    w_dt = din("w_dt", [2, DT_RANK, D_INNER], BF16)
    cwd = din("cwd", [2, 128, K_CONV, NG, 128], BF16)     # host-built diagonals
    cbh = din("cbh", [2, D_INNER])                        # 0.5*conv_b
    dtb = din("dtb", [2, D_INNER])
    An = din("An", [2, D_INNER, D_STATE])                 # -exp(A_log)
    dp = din("dp", [2, D_INNER])                          # host-scaled x0.5
    outT = nc.dram_tensor("outT", [D_MODEL, L], F32, kind="ExternalOutput").ap()

    bc_d = nc.dram_tensor("bc_d", [2 * D_STATE, L], BF16).ap()
    sz_d = nc.dram_tensor("sz_d", [D_INNER, L], BF16).ap()
    h1r_d = nc.dram_tensor("h1r_d", [D_MODEL, L], F32).ap()
    h1rb_d = nc.dram_tensor("h1rb_d", [D_MODEL, L], BF16).ap()

    dbg = {}
    if debug:
        for name, shape in [("dbg_xs", [D_INNER, L]),
                            ("dbg_dt", [D_INNER, L]), ("dbg_y", [D_INNER, L]),
                            ("dbg_xdbl", [56, L])]:
            dbg[name] = nc.dram_tensor(name, shape, F32, kind="ExternalOutput").ap()

    MMF = 512  # max free-dim columns per matmul (one PSUM bank of fp32)

    def mm(out, lhsT, rhs, first, last):
        """k-accumulating matmul, split into 512-column chunks."""
        F = rhs.shape[-1]
        for c in range(0, F, MMF):
            nc.tensor.matmul(out[:, c:c + MMF], lhsT, rhs[:, c:c + MMF],
                             start=first, stop=last)

    with tile.TileContext(nc) as tc, ExitStack() as ctx:
        consts = ctx.enter_context(tc.tile_pool(name="consts", bufs=1))
        ones_col = consts.tile([128, 1], BF16)
        nc.vector.memset(ones_col, 1.0)
        eye_t = consts.tile([128, 128], BF16)
        nc.sync.dma_start(out=eye_t, in_=eye)
        c_eps = consts.tile([1, 1], F32)
        nc.vector.memset(c_eps, EPS)

        for bid, blk in [(r * 2 + b, b) for r in range(repeat) for b in range(2)]:
            block_in = xT if blk == 0 else h1r_d
            block_in_bf = xTb if blk == 0 else h1rb_d
            block_out = h1r_d if blk == 0 else outT
            with ExitStack() as bctx:
                # ---- block-lifetime pools (stack order matters) ----
                wp = bctx.enter_context(tc.tile_pool(name=f"w{bid}", bufs=1))
                pers = bctx.enter_context(tc.tile_pool(name=f"pers{bid}", bufs=1))
                xs2 = pers.tile([128, NG, L], BF16)    # 2*silu(conv)
                ygt = pers.tile([128, NG, L], BF16)    # gated output
                x_dbl = pers.tile([56, L], BF16)
                bp = bctx.enter_context(tc.tile_pool(name=f"bp{bid}", bufs=1))
                dt6 = bp.tile([128, NG, L], BF16, tag="dt6")
                prep = bctx.enter_context(tc.tile_pool(name=f"prep{bid}", bufs=2))
                P1 = bctx.enter_context(tc.tile_pool(name=f"P1_{bid}", bufs=1, space="PSUM"))
                P2 = bctx.enter_context(tc.tile_pool(name=f"P2_{bid}", bufs=1, space="PSUM"))

                # ---- phase A scope ----
                actx = bctx.enter_context(ExitStack())
                ap_big = actx.enter_context(tc.tile_pool(name=f"abig{bid}", bufs=1))
                ap_row = actx.enter_context(tc.tile_pool(name=f"arow{bid}", bufs=1))
                ap_rep = actx.enter_context(tc.tile_pool(name=f"arep{bid}", bufs=1))
                ap_sm = actx.enter_context(tc.tile_pool(name=f"asm{bid}", bufs=2))

                # input first: everything hangs off rin, so its DMA must not
                # queue behind the weight loads
                rin = ap_big.tile([128, NM, L], BF16)
                for g in range(NM):
                    nc.sync.dma_start(out=rin[:, g, :],
                                      in_=block_in_bf[128 * g:128 * (g + 1), :])

                wo_t = wp.tile([128, NG, D_MODEL], BF16)
                nc.sync.dma_start(out=wo_t, in_=w_out[blk].rearrange("(k p) m -> p k m", p=128))
                wdt_t = wp.tile([DT_RANK, D_INNER], BF16)
                nc.sync.dma_start(out=wdt_t, in_=w_dt[blk])
                w4_t = wp.tile([2, 2 * D_INNER], BF16)
                nc.sync.dma_start(out=w4_t, in_=w4[blk])
                cbh_t = wp.tile([128, NG], F32)
                nc.sync.dma_start(out=cbh_t, in_=cbh[blk].rearrange("(g p) -> p g", p=128))
                dtb_t = wp.tile([128, NG], F32)
                nc.sync.dma_start(out=dtb_t, in_=dtb[blk].rearrange("(g p) -> p g", p=128))
                dp_t = wp.tile([128, NG], F32)
                nc.sync.dma_start(out=dp_t, in_=dp[blk].rearrange("(g p) -> p g", p=128))
                An_t = wp.tile([128, NG, D_STATE], F32)
                nc.sync.dma_start(out=An_t, in_=An[blk].rearrange("(g p) n -> p g n", p=128))
                wi_t = ap_big.tile([128, NM, 2 * D_INNER], BF16)
                nc.sync.dma_start(out=wi_t, in_=w_in[blk].rearrange("(k p) m -> p k m", p=128))
                wxp_t = ap_big.tile([128, NG, 56], BF16)
                nc.sync.dma_start(out=wxp_t, in_=w_xp[blk].rearrange("(k p) m -> p k m", p=128))
                diag_t = ap_big.tile([128, K_CONV, NG, 128], BF16)
                nc.sync.dma_start(out=diag_t, in_=cwd[blk])

                # LN stats: sum and sum-of-squares rows via ones-matmul
                mu_ps = P1.tile([1, L], F32, tag="A")
                for g in range(NM):
                    mm(mu_ps, ones_col, rin[:, g, :], g == 0, g == NM - 1)
                sq_ps = P2.tile([1, L], F32, tag="B")
                for g in range(NM):
                    sq = ap_sm.tile([128, L], BF16, tag="t1")
                    nc.vector.tensor_mul(sq, rin[:, g, :], rin[:, g, :])
                    mm(sq_ps, ones_col, sq, g == 0, g == NM - 1)
                rows2 = ap_row.tile([1, 2, L], BF16)
                mu_row = rows2[:, 0, :]
                var_row = rows2[:, 1, :]
                nc.scalar.activation(mu_row, mu_ps, AF.Identity, scale=1.0 / D_MODEL)
                nc.scalar.activation(var_row, sq_ps, AF.Identity, scale=1.0 / D_MODEL,
                                     bias=c_eps)
                # token-major reshape (t = 16p + c; any bijection works since
                # the rsqrt is elementwise) so Newton runs on 16-wide tiles
                LT16 = L // 128
                tmb2 = ap_row.tile([128, 2, LT16], BF16)
                for r in range(2):
                    nc.sync.dma_start(
                        out=tmb2[:, r, :],
                        in_=bass.AP(tensor=rows2.tensor, offset=rows2.offset + r * L,
                                    ap=[[rows2.ap[0][0], 1], [LT16, 128], [1, LT16]]))
                tm = ap_row.tile([128, 4, LT16], F32)
                mu_tm = tm[:, 0, :]
                var_tm = tm[:, 1, :]
                yr = tm[:, 2, :]
                tnw = tm[:, 3, :]
                nc.vector.tensor_copy(mu_tm, tmb2[:, 0, :])
                nc.vector.tensor_copy(var_tm, tmb2[:, 1, :])
                nc.vector.tensor_mul(tnw, mu_tm, mu_tm)
                nc.vector.tensor_sub(var_tm, var_tm, tnw)
                # rstd = rsqrt(var+eps) via bit-trick seed + 2 Newton steps
                I32 = mybir.dt.int32
                yi = yr.bitcast(I32)
                nc.vector.tensor_scalar(yi, var_tm.bitcast(I32), 1, None,
                                        OP.arith_shift_right)
                nc.vector.tensor_scalar(yi, yi, -1, 0x5f3759df, OP.mult, OP.add)
                for _ in range(2):
                    nc.vector.tensor_mul(tnw, yr, yr)
                    nc.vector.tensor_mul(tnw, tnw, var_tm)
                    nc.vector.tensor_scalar(tnw, tnw, -0.5, 1.5, OP.mult, OP.add)
                    nc.vector.tensor_mul(yr, yr, tnw)
                # rs -> row (reuse mu slot) for broadcast; [mu*rs; 1] -> ex2
                tmb = ap_row.tile([128, 2, LT16], BF16)
                nc.vector.tensor_copy(tmb[:, 0, :], yr)
                nc.vector.tensor_mul(mu_tm, mu_tm, yr)
                nc.vector.tensor_copy(tmb[:, 1, :], mu_tm)
                ex2 = ap_row.tile([2, L], BF16)
                nc.vector.memset(ex2, 1.0)
                nc.sync.dma_start(
                    out=bass.AP(tensor=rows2.tensor, offset=rows2.offset,
                                ap=[[rows2.ap[0][0], 1], [LT16, 128], [1, LT16]]),
                    in_=tmb[:, 0, :])
                nc.sync.dma_start(
                    out=bass.AP(tensor=ex2.tensor, offset=ex2.offset,
                                ap=[[ex2.ap[0][0], 1], [LT16, 128], [1, LT16]]),
                    in_=tmb[:, 1, :])
                rs_rep = ap_rep.tile([128, L], BF16, tag="rep_rs")
                nc.gpsimd.partition_broadcast(rs_rep, rows2[:, 0, :])
                for g in range(NM):
                    nc.vector.tensor_mul(rin[:, g, :], rin[:, g, :], rs_rep)

                # in_proj: 12 output feature tiles, LN fully folded in
                xsp = ap_big.tile([128, NG, K_CONV - 1 + L], BF16)
                nc.vector.memset(xsp[:, :, 0:K_CONV - 1], 0.0)
                for f in range(2 * NG):
                    ps = (P1 if f % 2 == 0 else P2).tile(
                        [128, L], F32, tag="A" if f % 2 == 0 else "B")
                    for k in range(NM):
                        mm(ps, wi_t[:, k, 128 * f:128 * (f + 1)], rin[:, k, :],
                           k == 0, False)
                    mm(ps, w4_t[:, 128 * f:128 * (f + 1)], ex2, False, True)
                    if f < NG:
                        nc.scalar.copy(xsp[:, f, K_CONV - 1:], ps)
                    else:
                        g = f - NG
                        zt = ap_sm.tile([128, L], BF16, tag="t2")
                        nc.scalar.copy(zt, ps)
                        tz = ap_sm.tile([128, L], BF16, tag="t3")
                        nc.scalar.activation(tz, zt, AF.Tanh, scale=0.5)
                        uz = ap_sm.tile([128, L], BF16, tag="t1")
                        nc.vector.tensor_scalar_add(uz, tz, 1.0)
                        szt = ap_sm.tile([128, L], BF16, tag="t4")
                        nc.vector.tensor_mul(szt, uz, zt)
                        nc.sync.dma_start(out=sz_d[128 * g:128 * (g + 1), :], in_=szt)

                # conv on PE (diag matmuls, P2) + tanh-silu + x_proj accum (P1)
                px = P1.tile([56, L], F32, tag="A")
                for g in range(NG):
                    ps2 = P2.tile([128, L], F32, tag="B")
                    for j in range(K_CONV):
                        mm(ps2, diag_t[:, j, g, :], xsp[:, g, j:j + L],
                           j == 0, j == K_CONV - 1)
                    cfh = ap_sm.tile([128, L], BF16, tag="t2")
                    nc.scalar.activation(cfh, ps2, AF.Identity, scale=0.5,
                                         bias=cbh_t[:, g:g + 1])
                    tc_t = ap_sm.tile([128, L], BF16, tag="t3")
                    nc.scalar.activation(tc_t, cfh, AF.Tanh)
                    u2 = ap_sm.tile([128, L], BF16, tag="t1")
                    nc.vector.tensor_scalar(u2, tc_t, 2.0, 2.0, OP.mult, OP.add)
                    nc.vector.tensor_mul(xs2[:, g, :], u2, cfh)
                    mm(px, wxp_t[:, g, :], xs2[:, g, :], g == 0, g == NG - 1)
                    if debug and blk == 0:
                        nc.gpsimd.dma_start(out=dbg["dbg_xs"][128 * g:128 * (g + 1), :], in_=xs2[:, g, :])
                nc.scalar.copy(x_dbl, px)
                nc.sync.dma_start(out=bc_d, in_=x_dbl[DT_RANK:DT_RANK + 2 * D_STATE, :])
                if debug and blk == 0:
                    nc.gpsimd.dma_start(out=dbg["dbg_xdbl"], in_=x_dbl)

                # dt for all groups (softplus(z) = u*(1-u/2), u=e^z, z<=-3.5)
                for g in range(NG):
                    psd = (P1 if g % 2 == 0 else P2).tile(
                        [128, L], F32, tag="A" if g % 2 == 0 else "B")
                    mm(psd, wdt_t[:, 128 * g:128 * (g + 1)], x_dbl[0:DT_RANK, :],
                       True, True)
                    uu = ap_sm.tile([128, L], BF16, tag="t2")
                    nc.scalar.activation(uu, psd, AF.Exp, bias=dtb_t[:, g:g + 1])
                    t0 = ap_sm.tile([128, L], BF16, tag="t3")
                    nc.vector.tensor_scalar(t0, uu, -0.5, 1.0, OP.mult, OP.add)
                    nc.vector.tensor_mul(dt6[:, g, :], t0, uu)
                    if debug and blk == 0:
                        nc.gpsimd.dma_start(out=dbg["dbg_dt"][128 * g:128 * (g + 1), :], in_=dt6[:, g, :])

                # prefetch first broadcast loads before the pool-release barrier
                pre_reps = []
                for n in range(PREF):
                    brep = prep.tile([128, L], BF16, tag="brep")
                    nc.sync.dma_start(out=brep, in_=_bcast_row(bc_d[n:n + 1, :]))
                    crep = prep.tile([128, L], BF16, tag="crep")
                    nc.sync.dma_start(out=crep, in_=_bcast_row(bc_d[D_STATE + n:D_STATE + n + 1, :]))
                    pre_reps.append((brep, crep))

                actx.close()   # release phase-A pools (one barrier)

                # ---------------- Phase B: scan ------------------------------
                with ExitStack() as sctx:
                    yp3 = sctx.enter_context(tc.tile_pool(name=f"yp{bid}", bufs=1))
                    stg = sctx.enter_context(tc.tile_pool(name=f"stg{bid}", bufs=2))
                    sc = sctx.enter_context(tc.tile_pool(name=f"sc{bid}", bufs=sc_bufs))
                    for gh in range(NG // GH):
                        dtx3 = yp3.tile([128, GH, L], BF16, tag="dtx3",
                                        name=f"dtx3_{bid}_{gh}")
                        y3 = yp3.tile([128, GH, L], BF16, tag="y3",
                                      name=f"y3_{bid}_{gh}")
                        for gi in range(GH):
                            g = gh * GH + gi
                            nc.vector.tensor_mul(dtx3[:, gi, :], dt6[:, g, :], xs2[:, g, :])
                        y_ps = []
                        for gi in range(GH):
                            yp = (P1 if gi == 0 else P2).tile(
                                [128, L], F32, tag="A" if gi == 0 else "B",
                                name=f"yps{bid}_{gh}_{gi}")
                            y_ps.append(yp)
                        for n in range(D_STATE):
                            if gh == 0 and n < PREF:
                                brep, crep = pre_reps[n]
                            else:
                                brep = prep.tile([128, L], BF16, tag="brep",
                                                 name=f"brep{bid}_{gh}_{n}")
                                nc.sync.dma_start(out=brep, in_=_bcast_row(bc_d[n:n + 1, :]))
                                crep = prep.tile([128, L], BF16, tag="crep",
                                                 name=f"crep{bid}_{gh}_{n}")
                                nc.sync.dma_start(out=crep, in_=_bcast_row(bc_d[D_STATE + n:D_STATE + n + 1, :]))
                            abh = []
                            for gi in range(GH):
                                g = gh * GH + gi
                                a = sc.tile([128, L], BF16, tag="a",
                                            name=f"a{bid}_{gh}_{n}_{gi}")
                                nc.scalar.activation(a, dt6[:, g, :], AF.Exp,
                                                     scale=An_t[:, g, n:n + 1])
                                abh.append([a, None])
                            for gi in range(GH):
                                slot = n * GH + gi
                                beng = nc.vector if (slot % 4) < b_dve else nc.gpsimd
                                b = sc.tile([128, L], BF16, tag="b",
                                            name=f"b{bid}_{gh}_{n}_{gi}")
                                beng.tensor_mul(b, dtx3[:, gi, :], brep)
                                abh[gi][1] = b
                            hs = []
                            for gi in range(GH):
                                h = sc.tile([128, L], BF16, tag="h",
                                            name=f"h{bid}_{gh}_{n}_{gi}")
                                nc.vector.tensor_tensor_scan(
                                    h, abh[gi][0], abh[gi][1], 0.0, OP.mult, OP.add)
                                hs.append(h)
                            for gi in range(GH):
                                slot = n * GH + gi
                                heng = nc.vector if ((slot + 2) % 4) < hc_dve else nc.gpsimd
                                hc = sc.tile([128, L], BF16, tag="hc",
                                             name=f"hc{bid}_{gh}_{n}_{gi}")
                                heng.tensor_mul(hc, hs[gi], crep)
                                # y += hc via identity matmul (PSUM accumulate)
                                mm(y_ps[gi], eye_t, hc, n == 0, n == D_STATE - 1)
                        for gi in range(GH):
                            g = gh * GH + gi
                            y3s = yp3.tile([128, L], BF16, tag=f"y3s{gi}",
                                           name=f"y3s{bid}_{gh}_{gi}")
                            nc.scalar.copy(y3s, y_ps[gi])
                            if debug and blk == 0:
                                nc.gpsimd.dma_start(out=dbg["dbg_y"][128 * g:128 * (g + 1), :],
                                                    in_=y3s)
                            # gate: ygt = (xs2*dp' + y) * sz2
                            szg = stg.tile([128, L], BF16, tag="szg",
                                           name=f"szg{bid}_{gh}_{gi}")
                            nc.sync.dma_start(out=szg, in_=sz_d[128 * g:128 * (g + 1), :])
                            xsd = stg.tile([128, L], BF16, tag="xsd",
                                           name=f"xsd{bid}_{gh}_{gi}")
                            nc.vector.tensor_scalar(xsd, xs2[:, g, :],
                                                    dp_t[:, g:g + 1], None, OP.mult)
                            tmp = stg.tile([128, L], BF16, tag="gt",
                                           name=f"gt{bid}_{gh}_{gi}")
                            getattr(nc, gate_eng).tensor_add(tmp, xsd, y3s)
                            nc.vector.tensor_mul(ygt[:, g, :], tmp, szg)

                # -------- Phase C: out_proj + residual + reversed store ------
                with ExitStack() as cctx:
                    cp = cctx.enter_context(tc.tile_pool(name=f"cp{bid}", bufs=2))
                    rin2 = cctx.enter_context(tc.tile_pool(name=f"rin2{bid}", bufs=1)) \
                        .tile([128, NM, L], F32)
                    nc.sync.dma_start(out=rin2, in_=block_in.rearrange("(i p) t -> p i t", p=128))
                    reng = getattr(nc, resid_eng)
                    pi = 0
                    for dm in range(NM):
                        ho = cp.tile([128, L], F32, tag="ho")
                        for c in range(0, L, MMF):
                            pso = (P1 if pi % 2 == 0 else P2).tile(
                                [128, MMF], F32, tag="A" if pi % 2 == 0 else "B",
                                name=f"pso{bid}_{dm}_{c}")
                            pi += 1
                            for g in range(NG):
                                nc.tensor.matmul(pso, wo_t[:, g, 128 * dm:128 * (dm + 1)],
                                                 ygt[:, g, c:c + MMF],
                                                 start=(g == 0), stop=(g == NG - 1))
                            reng.tensor_add(ho[:, c:c + MMF], pso, rin2[:, dm, c:c + MMF])
                        hr = cp.tile([128, L], F32, tag="hr")
                        nc.vector.tensor_copy(hr, ho[:, ::-1])
                        nc.sync.dma_start(out=block_out[128 * dm:128 * (dm + 1), :], in_=hr)
                        if blk == 0:
                            hrb = cp.tile([128, L], BF16, tag="hrb")
                            nc.vector.tensor_copy(hrb, hr)
                            nc.sync.dma_start(out=h1rb_d[128 * dm:128 * (dm + 1), :], in_=hrb)

    nc.compile()
    return nc


_NC_CACHE = {}


def _get_nc(debug=False):
    if debug not in _NC_CACHE:
        _NC_CACHE[debug] = build_module(debug)
    return _NC_CACHE[debug]


def prep_host(inputs):
    """Host-side weight prep shared by all cores.

    Folds: silu computed as x*(tanh(x/2)+1) = 2*silu(x) on-device, so
      - x_proj rows all get x0.5 (xs2 = 2*silu_true); C rows get another
        x0.5 (the scan's y is 2x true because dtx2 = 2*dtx_true)
      - Dp gets x0.5 (skip term uses xs2)
      - out_proj gets x0.5 (gate uses sz2 = 2*silu_true(z))
    LN folded into in_proj (W' = w_in^T * nw plus [-S; c] extra rows).
    Conv weights become per-tap diagonal matrices for the PE.
    """
    import ml_dtypes
    f = np.float32
    bf = ml_dtypes.bfloat16
    cw = np.ascontiguousarray(inputs["conv_w"][:, :, 0, :]).astype(f)  # (2,768,4)
    diag = np.zeros((2, 128, K_CONV, NG, 128), f)
    for g in range(NG):
        blkw = cw[:, g * 128:(g + 1) * 128, :]          # (2,128,4)
        idx = np.arange(128)
        diag[:, idx, :, g, idx] = np.transpose(blkw, (1, 0, 2))  # (128,2,4)
    xp = np.ascontiguousarray(np.transpose(inputs["x_proj"], (0, 2, 1))).astype(f)
    xp = xp * 0.5
    xp[:, :, DT_RANK + D_STATE:] *= 0.5                 # C columns: x0.25 total
    wiT = np.ascontiguousarray(np.transpose(inputs["in_proj"], (0, 2, 1))).astype(f)
    wiT = wiT * inputs["norm_w"].astype(f)[:, :, None]
    s_in = wiT.sum(axis=1)                              # (2, 1536)
    wiT_raw = np.ascontiguousarray(np.transpose(inputs["in_proj"], (0, 2, 1))).astype(f)
    c_all = np.einsum('bm,bmf->bf', inputs["norm_b"].astype(f), wiT_raw)
    w4 = np.stack([-s_in, c_all], axis=1)               # (2, 2, 1536)
    return {
        "eye": np.eye(128, dtype=f).astype(bf),
        "w_in": wiT.astype(bf),
        "w4": w4.astype(bf),
        "w_out": (np.ascontiguousarray(np.transpose(inputs["out_proj"], (0, 2, 1))) * 0.5).astype(bf),
        "w_xp": xp.astype(bf),
        "w_dt": np.ascontiguousarray(np.transpose(inputs["dt_w"], (0, 2, 1))).astype(bf),
        "cwd": diag.astype(bf),
        "cbh": (0.5 * inputs["conv_b"]).astype(f),
        "dtb": inputs["dt_b"].astype(f),
        "An": (-np.exp(inputs["A_log"])).astype(f),
        "dp": (0.5 * inputs["Dp"]).astype(f),
    }


def build_module_repeat(k):
    return build_module(False, repeat=k)


def kernel(**inputs):
    inputs = {k: np.asarray(v) for k, v in inputs.items()}
    nc = _get_nc(False)
    shared = prep_host(inputs)
    import ml_dtypes
    in_maps = []
    for s in range(BATCH):
        m = dict(shared)
        xt = np.ascontiguousarray(inputs["x"][s].T).astype(np.float32)
        m["xT"] = xt
        m["xTb"] = xt.astype(ml_dtypes.bfloat16)
        in_maps.append(m)
    res = run_bass_kernel_spmd(nc, in_maps, list(range(BATCH)))
    out = np.stack([res.results[s]["outT"].T for s in range(BATCH)])
    return np.ascontiguousarray(out.astype(np.float32))



# revision 38
# speedup vs baseline: 1.3935x; 1.3935x over previous
"""Bi-Mamba Trainium2 kernel (v4.1 — HW-calibrated engine rebalance).

Changes vs v4 baseline, driven by slope-microbenchmark HW calibration:
  - Pool/gpsimd tensor_tensor muls measure ~4.0us per [128,2048] bf16 tile
    (no bf16 packing on the Q7 cores; ~4x slower than DVE's 1.0us) and DVE
    tensor_tensor_scan measures 2 cyc/elem (~4.3us, dtype-independent), so
    the v4 Pool-heavy scan assignment made Pool the bottleneck. The b/hc
    broadcast muls are rebalanced (b_dve=2, hc_dve=1 of 4 slots to DVE)
    and the gate add moved to DVE.
  - A single [1->128]-partition broadcast DMA measures ~4.0us (~130GB/s per
    hardware queue); the scan issues 96 of them per block plus sz/weight/IO
    traffic, so a single SP queue (~67MB/block) was co-limiting. DMAs are
    now spread across queues: C-row broadcasts issue from the ACT hwdge
    queue, sz gate loads stay on SP, phase-C residual loads on ACT.
    (Routing sz loads through gpsimd SWDGE measured worse - descriptor
    generation steals Pool engine time; extra ACT compute ops also stall
    ACT-queue DMA issue, so ACT carries DMAs but no extra compute.)
Round-interleaved HW A/B: spread+rebalance ~0.9-1.2ms vs ~1.5-1.9ms for
rebalance-only vs ~2.6-2.9ms for v4, same-session units. An NS=2
state-blocked scan variant (v5, cfg-gated) validated numerically but
measured slower on HW (stride-0 replicated operands defeat DVE 2x packing).


Contract: kernel(**inputs) takes the FULL unsharded inputs (numpy) keyed as
reference.setup_inputs() and returns the FULL (8, 2048, 384) float32 output.

Sharding: pure data-parallel over batch — 8 samples, 8 cores, one sample per
core, no collectives. All weights are replicated per core.

Design notes (per core, channel-major [feature, token] layout):
  - LayerNorm is folded into the in_proj matmul: stats via PE ones-matmuls,
    rsqrt via bit-trick+Newton on token-major-reshaped 16-wide tiles, then
    the matmul input is x*rs and two extra contraction rows [mu*rs; 1] with
    host-prepared weights [-S[f]; c[f]] complete the normalized projection.
  - silu(x) is computed as x*(tanh(x/2)+1) = 2*silu(x) with the 0.5 folded
    into host-prepped weights (x_proj rows, Dp, out_proj): ACT tanh + one
    4x-mode tensor_scalar + one 2x-mode bf16 mul, no 1x-mode STT ops.
  - causal depthwise conv (k=4) runs on the PE as 4 diagonal-matrix matmuls
    accumulated in PSUM (diagonals built on the host).
  - everything that streams is bf16 (DVE TensorTensor is 2x only for 2-byte
    packed dtypes; matmuls are full rate in bf16).
  - scan: per (128-channel group, state): a = exp(A*dt) on ACT, b = dtx*B_rep
    (DVE/Pool), h = tensor_tensor_scan(a,b) [DVE-only, 1x], hc = h*C_rep
    (DVE/Pool), and y += hc happens on the PE as an identity-matmul PSUM
    accumulation (GH=2 groups in flight, 2x4 PSUM banks).
  - out_proj accumulates over the 6 channel groups in PSUM, then residual-add
    and a reversed store so the second block runs identical code on the
    flipped sequence.
  - two shared block-scope PSUM pools and a single scan scope avoid
    pool-release barriers inside the scan; the first broadcast DMAs are
    prefetched before the phase-A pool release barrier.
"""
import numpy as np
from contextlib import ExitStack

import concourse.bass as bass
import concourse.tile as tile
from concourse import bacc, mybir
from concourse.bass_utils import run_bass_kernel_spmd

F32 = mybir.dt.float32
BF16 = mybir.dt.bfloat16
AF = mybir.ActivationFunctionType
OP = mybir.AluOpType

D_MODEL = 384
D_INNER = 768
D_STATE = 16
DT_RANK = 24
K_CONV = 4
L = 2048
BATCH = 8
EPS = 1e-5
NG = D_INNER // 128      # 6 channel blocks of d_inner
NM = D_MODEL // 128      # 3 channel blocks of d_model


def _bcast_row(ap):
    """View a [1, L] AP as [128, L] with partition step 0 (replicated read)."""
    return bass.AP(tensor=ap.tensor, offset=ap.offset, ap=[[0, 128]] + list(ap.ap[1:]))


def _bcast_rows(ap):
    """View an [R, L] DRAM AP as [128, R, L] with partition step 0."""
    return bass.AP(tensor=ap.tensor, offset=ap.offset, ap=[[0, 128]] + list(ap.ap))


def build_module(debug=False, repeat=1, cfg=None):
    cfg = cfg or {}
    b_dve = cfg.get('b_dve', 2)        # of 4 slots, how many b-muls go to DVE
    hc_dve = cfg.get('hc_dve', 1)
    gate_eng = cfg.get('gate_eng', 'vector')
    resid_eng = cfg.get('resid_eng', 'vector')
    GH = 2
    sc_bufs = cfg.get('sc_bufs', 2)
    PREF = cfg.get('pref', 2)          # broadcast loads prefetched pre-barrier
    skip_scan = cfg.get('skip_scan', False)   # timing ablation only
    skip_c = cfg.get('skip_c', False)         # timing ablation only
    v5 = cfg.get('v5', False)                 # state-blocked scan (slower on HW)
    NS = cfg.get('ns', 2)                     # states per tts op (v5)
    xsd_act = cfg.get('xsd_act', False)       # xsd = xs2*dp' on ACT instead of DVE
    dma_spread = cfg.get('dma_spread', True)   # crep via ACT queue, szg via SWDGE
    sz_sbuf = cfg.get('sz_sbuf', False)        # keep sz2 in SBUF (no DRAM roundtrip)
    sz_sp = cfg.get('sz_sp', True)             # with dma_spread: szg stays on SP

    nc = bacc.Bacc("TRN2", target_bir_lowering=False, debug=False)

    def din(name, shape, dt=F32):
        return nc.dram_tensor(name, shape, dt, kind="ExternalInput").ap()

    xT = din("xT", [D_MODEL, L])
    xTb = din("xTb", [D_MODEL, L], BF16)
    eye = din("eye", [128, 128], BF16)
    w_in = din("w_in", [2, D_MODEL, 2 * D_INNER], BF16)   # host: folded x norm_w
    w4 = din("w4", [2, 2, 2 * D_INNER], BF16)             # [-S[f]; c[f]] rows
    w_out = din("w_out", [2, D_INNER, D_MODEL], BF16)     # host-scaled x0.5
    w_xp = din("w_xp", [2, D_INNER, 56], BF16)            # host-scaled (see prep)
    w_dt = din("w_dt", [2, DT_RANK, D_INNER], BF16)
    cwd = din("cwd", [2, 128, K_CONV, NG, 128], BF16)     # host-built diagonals
    cbh = din("cbh", [2, D_INNER])                        # 0.5*conv_b
    dtb = din("dtb", [2, D_INNER])
    An = din("An", [2, D_INNER, D_STATE])                 # -exp(A_log)
    dp = din("dp", [2, D_INNER])                          # host-scaled x0.5
    outT = nc.dram_tensor("outT", [D_MODEL, L], F32, kind="ExternalOutput").ap()

    bc_d = nc.dram_tensor("bc_d", [2 * D_STATE, L], BF16).ap()
    sz_d = nc.dram_tensor("sz_d", [D_INNER, L], BF16).ap()
    h1r_d = nc.dram_tensor("h1r_d", [D_MODEL, L], F32).ap()
    h1rb_d = nc.dram_tensor("h1rb_d", [D_MODEL, L], BF16).ap()

    dbg = {}
    if debug:
        for name, shape in [("dbg_xs", [D_INNER, L]),
                            ("dbg_dt", [D_INNER, L]), ("dbg_y", [D_INNER, L]),
                            ("dbg_xdbl", [56, L])]:
            dbg[name] = nc.dram_tensor(name, shape, F32, kind="ExternalOutput").ap()

    MMF = 512  # max free-dim columns per matmul (one PSUM bank of fp32)

    def mm(out, lhsT, rhs, first, last):
        """k-accumulating matmul, split into 512-column chunks."""
        F = rhs.shape[-1]
        for c in range(0, F, MMF):
            nc.tensor.matmul(out[:, c:c + MMF], lhsT, rhs[:, c:c + MMF],
                             start=first, stop=last)

    with tile.TileContext(nc) as tc, ExitStack() as ctx:
        consts = ctx.enter_context(tc.tile_pool(name="consts", bufs=1))
        ones_col = consts.tile([128, 1], BF16)
        nc.vector.memset(ones_col, 1.0)
        eye_t = consts.tile([128, 128], BF16)
        nc.sync.dma_start(out=eye_t, in_=eye)
        c_eps = consts.tile([1, 1], F32)
        nc.vector.memset(c_eps, EPS)

        for bid, blk in [(r * 2 + b, b) for r in range(repeat) for b in range(2)]:
            block_in = xT if blk == 0 else h1r_d
            block_in_bf = xTb if blk == 0 else h1rb_d
            block_out = h1r_d if blk == 0 else outT
            with ExitStack() as bctx:
                # ---- block-lifetime pools (stack order matters) ----
                wp = bctx.enter_context(tc.tile_pool(name=f"w{bid}", bufs=1))
                pers = bctx.enter_context(tc.tile_pool(name=f"pers{bid}", bufs=1))
                xs2 = pers.tile([128, NG, L], BF16)    # 2*silu(conv)
                ygt = pers.tile([128, NG, L], BF16)    # gated output
                x_dbl = pers.tile([56, L], BF16)
                szp = pers.tile([128, NG, L], BF16, name=f"szp{bid}") if sz_sbuf else None
                bp = bctx.enter_context(tc.tile_pool(name=f"bp{bid}", bufs=1))
                dt6 = bp.tile([128, NG, L], BF16, tag="dt6")
                prep = bctx.enter_context(tc.tile_pool(name=f"prep{bid}", bufs=2))
                P1 = bctx.enter_context(tc.tile_pool(name=f"P1_{bid}", bufs=1, space="PSUM"))
                P2 = bctx.enter_context(tc.tile_pool(name=f"P2_{bid}", bufs=1, space="PSUM"))

                # ---- phase A scope ----
                actx = bctx.enter_context(ExitStack())
                ap_big = actx.enter_context(tc.tile_pool(name=f"abig{bid}", bufs=1))
                ap_row = actx.enter_context(tc.tile_pool(name=f"arow{bid}", bufs=1))
                ap_rep = actx.enter_context(tc.tile_pool(name=f"arep{bid}", bufs=1))
                ap_sm = actx.enter_context(tc.tile_pool(name=f"asm{bid}", bufs=2))

                # input first: everything hangs off rin, so its DMA must not
                # queue behind the weight loads
                rin = ap_big.tile([128, NM, L], BF16)
                for g in range(NM):
                    nc.sync.dma_start(out=rin[:, g, :],
                                      in_=block_in_bf[128 * g:128 * (g + 1), :])

                wo_t = wp.tile([128, NG, D_MODEL], BF16)
                nc.sync.dma_start(out=wo_t, in_=w_out[blk].rearrange("(k p) m -> p k m", p=128))
                wdt_t = wp.tile([DT_RANK, D_INNER], BF16)
                nc.sync.dma_start(out=wdt_t, in_=w_dt[blk])
                w4_t = wp.tile([2, 2 * D_INNER], BF16)
                nc.sync.dma_start(out=w4_t, in_=w4[blk])
                cbh_t = wp.tile([128, NG], F32)
                nc.sync.dma_start(out=cbh_t, in_=cbh[blk].rearrange("(g p) -> p g", p=128))
                dtb_t = wp.tile([128, NG], F32)
                nc.sync.dma_start(out=dtb_t, in_=dtb[blk].rearrange("(g p) -> p g", p=128))
                dp_t = wp.tile([128, NG], F32)
                nc.sync.dma_start(out=dp_t, in_=dp[blk].rearrange("(g p) -> p g", p=128))
                An_t = wp.tile([128, NG, D_STATE], F32)
                nc.sync.dma_start(out=An_t, in_=An[blk].rearrange("(g p) n -> p g n", p=128))
                wi_t = ap_big.tile([128, NM, 2 * D_INNER], BF16)
                nc.sync.dma_start(out=wi_t, in_=w_in[blk].rearrange("(k p) m -> p k m", p=128))
                wxp_t = ap_big.tile([128, NG, 56], BF16)
                nc.sync.dma_start(out=wxp_t, in_=w_xp[blk].rearrange("(k p) m -> p k m", p=128))
                diag_t = ap_big.tile([128, K_CONV, NG, 128], BF16)
                nc.sync.dma_start(out=diag_t, in_=cwd[blk])

                # LN stats: sum and sum-of-squares rows via ones-matmul
                mu_ps = P1.tile([1, L], F32, tag="A")
                for g in range(NM):
                    mm(mu_ps, ones_col, rin[:, g, :], g == 0, g == NM - 1)
                sq_ps = P2.tile([1, L], F32, tag="B")
                for g in range(NM):
                    sq = ap_sm.tile([128, L], BF16, tag="t1")
                    nc.vector.tensor_mul(sq, rin[:, g, :], rin[:, g, :])
                    mm(sq_ps, ones_col, sq, g == 0, g == NM - 1)
                rows2 = ap_row.tile([1, 2, L], BF16)
                mu_row = rows2[:, 0, :]
                var_row = rows2[:, 1, :]
                nc.scalar.activation(mu_row, mu_ps, AF.Identity, scale=1.0 / D_MODEL)
                nc.scalar.activation(var_row, sq_ps, AF.Identity, scale=1.0 / D_MODEL,
                                     bias=c_eps)
                # token-major reshape (t = 16p + c; any bijection works since
                # the rsqrt is elementwise) so Newton runs on 16-wide tiles
                LT16 = L // 128
                tmb2 = ap_row.tile([128, 2, LT16], BF16)
                for r in range(2):
                    nc.sync.dma_start(
                        out=tmb2[:, r, :],
                        in_=bass.AP(tensor=rows2.tensor, offset=rows2.offset + r * L,
                                    ap=[[rows2.ap[0][0], 1], [LT16, 128], [1, LT16]]))
                tm = ap_row.tile([128, 4, LT16], F32)
                mu_tm = tm[:, 0, :]
                var_tm = tm[:, 1, :]
                yr = tm[:, 2, :]
                tnw = tm[:, 3, :]
                nc.vector.tensor_copy(mu_tm, tmb2[:, 0, :])
                nc.vector.tensor_copy(var_tm, tmb2[:, 1, :])
                nc.vector.tensor_mul(tnw, mu_tm, mu_tm)
                nc.vector.tensor_sub(var_tm, var_tm, tnw)
                # rstd = rsqrt(var+eps) via bit-trick seed + 2 Newton steps
                I32 = mybir.dt.int32
                yi = yr.bitcast(I32)
                nc.vector.tensor_scalar(yi, var_tm.bitcast(I32), 1, None,
                                        OP.arith_shift_right)
                nc.vector.tensor_scalar(yi, yi, -1, 0x5f3759df, OP.mult, OP.add)
                for _ in range(2):
                    nc.vector.tensor_mul(tnw, yr, yr)
                    nc.vector.tensor_mul(tnw, tnw, var_tm)
                    nc.vector.tensor_scalar(tnw, tnw, -0.5, 1.5, OP.mult, OP.add)
                    nc.vector.tensor_mul(yr, yr, tnw)
                # rs -> row (reuse mu slot) for broadcast; [mu*rs; 1] -> ex2
                tmb = ap_row.tile([128, 2, LT16], BF16)
                nc.vector.tensor_copy(tmb[:, 0, :], yr)
                nc.vector.tensor_mul(mu_tm, mu_tm, yr)
                nc.vector.tensor_copy(tmb[:, 1, :], mu_tm)
                ex2 = ap_row.tile([2, L], BF16)
                nc.vector.memset(ex2, 1.0)
                nc.sync.dma_start(
                    out=bass.AP(tensor=rows2.tensor, offset=rows2.offset,
                                ap=[[rows2.ap[0][0], 1], [LT16, 128], [1, LT16]]),
                    in_=tmb[:, 0, :])
                nc.sync.dma_start(
                    out=bass.AP(tensor=ex2.tensor, offset=ex2.offset,
                                ap=[[ex2.ap[0][0], 1], [LT16, 128], [1, LT16]]),
                    in_=tmb[:, 1, :])
                rs_rep = ap_rep.tile([128, L], BF16, tag="rep_rs")
                nc.gpsimd.partition_broadcast(rs_rep, rows2[:, 0, :])
                for g in range(NM):
                    nc.vector.tensor_mul(rin[:, g, :], rin[:, g, :], rs_rep)

                # in_proj: 12 output feature tiles, LN fully folded in
                xsp = ap_big.tile([128, NG, K_CONV - 1 + L], BF16)
                nc.vector.memset(xsp[:, :, 0:K_CONV - 1], 0.0)
                for f in range(2 * NG):
                    ps = (P1 if f % 2 == 0 else P2).tile(
                        [128, L], F32, tag="A" if f % 2 == 0 else "B")
                    for k in range(NM):
                        mm(ps, wi_t[:, k, 128 * f:128 * (f + 1)], rin[:, k, :],
                           k == 0, False)
                    mm(ps, w4_t[:, 128 * f:128 * (f + 1)], ex2, False, True)
                    if f < NG:
                        nc.scalar.copy(xsp[:, f, K_CONV - 1:], ps)
                    else:
                        g = f - NG
                        zt = ap_sm.tile([128, L], BF16, tag="t2")
                        nc.scalar.copy(zt, ps)
                        tz = ap_sm.tile([128, L], BF16, tag="t3")
                        nc.scalar.activation(tz, zt, AF.Tanh, scale=0.5)
                        uz = ap_sm.tile([128, L], BF16, tag="t1")
                        nc.vector.tensor_scalar_add(uz, tz, 1.0)
                        if sz_sbuf:
                            nc.vector.tensor_mul(szp[:, g, :], uz, zt)
                        else:
                            szt = ap_sm.tile([128, L], BF16, tag="t4")
                            nc.vector.tensor_mul(szt, uz, zt)
                            nc.sync.dma_start(out=sz_d[128 * g:128 * (g + 1), :], in_=szt)

                # conv on PE (diag matmuls, P2) + tanh-silu + x_proj accum (P1)
                px = P1.tile([56, L], F32, tag="A")
                for g in range(NG):
                    ps2 = P2.tile([128, L], F32, tag="B")
                    for j in range(K_CONV):
                        mm(ps2, diag_t[:, j, g, :], xsp[:, g, j:j + L],
                           j == 0, j == K_CONV - 1)
                    cfh = ap_sm.tile([128, L], BF16, tag="t2")
                    nc.scalar.activation(cfh, ps2, AF.Identity, scale=0.5,
                                         bias=cbh_t[:, g:g + 1])
                    tc_t = ap_sm.tile([128, L], BF16, tag="t3")
                    nc.scalar.activation(tc_t, cfh, AF.Tanh)
                    u2 = ap_sm.tile([128, L], BF16, tag="t1")
                    nc.vector.tensor_scalar(u2, tc_t, 2.0, 2.0, OP.mult, OP.add)
                    nc.vector.tensor_mul(xs2[:, g, :], u2, cfh)
                    mm(px, wxp_t[:, g, :], xs2[:, g, :], g == 0, g == NG - 1)
                    if debug and blk == 0:
                        nc.gpsimd.dma_start(out=dbg["dbg_xs"][128 * g:128 * (g + 1), :], in_=xs2[:, g, :])
                nc.scalar.copy(x_dbl, px)
                nc.sync.dma_start(out=bc_d, in_=x_dbl[DT_RANK:DT_RANK + 2 * D_STATE, :])
                if debug and blk == 0:
                    nc.gpsimd.dma_start(out=dbg["dbg_xdbl"], in_=x_dbl)

                # dt for all groups (softplus(z) = u*(1-u/2), u=e^z, z<=-3.5)
                for g in range(NG):
                    psd = (P1 if g % 2 == 0 else P2).tile(
                        [128, L], F32, tag="A" if g % 2 == 0 else "B")
                    mm(psd, wdt_t[:, 128 * g:128 * (g + 1)], x_dbl[0:DT_RANK, :],
                       True, True)
                    uu = ap_sm.tile([128, L], BF16, tag="t2")
                    nc.scalar.activation(uu, psd, AF.Exp, bias=dtb_t[:, g:g + 1])
                    t0 = ap_sm.tile([128, L], BF16, tag="t3")
                    nc.vector.tensor_scalar(t0, uu, -0.5, 1.0, OP.mult, OP.add)
                    nc.vector.tensor_mul(dt6[:, g, :], t0, uu)
                    if debug and blk == 0:
                        nc.gpsimd.dma_start(out=dbg["dbg_dt"][128 * g:128 * (g + 1), :], in_=dt6[:, g, :])

                # prefetch first broadcast loads before the pool-release barrier
                pre_reps = []
                if skip_scan:
                    PREF = 0
                if v5:
                    pass   # broadcasts allocated inside the scan scope
                else:
                    for n in range(PREF):
                        brep = prep.tile([128, L], BF16, tag="brep")
                        nc.sync.dma_start(out=brep, in_=_bcast_row(bc_d[n:n + 1, :]))
                        crep = prep.tile([128, L], BF16, tag="crep")
                        nc.sync.dma_start(out=crep, in_=_bcast_row(bc_d[D_STATE + n:D_STATE + n + 1, :]))
                        pre_reps.append((brep, crep))

                actx.close()   # release phase-A pools (one barrier)

                # ---------------- Phase B: scan ------------------------------
                if skip_scan:
                    nc.vector.memset(ygt, 0.0)
                with ExitStack() as sctx:
                    yp3 = sctx.enter_context(tc.tile_pool(name=f"yp{bid}", bufs=1))
                    stg = sctx.enter_context(tc.tile_pool(name=f"stg{bid}", bufs=2))
                    sc = sctx.enter_context(tc.tile_pool(name=f"sc{bid}", bufs=sc_bufs))
                    if v5:
                        prep = sctx.enter_context(tc.tile_pool(name=f"prep2_{bid}", bufs=2))
                    NSG = D_STATE // NS
                    for gh in (range(NG // GH) if (v5 and not skip_scan) else []):
                        # ---- v5: state-blocked scan (NS states per op) ----
                        dtx3 = yp3.tile([128, GH, L], BF16, tag="dtx3",
                                        name=f"dtx3_{bid}_{gh}")
                        for gi in range(GH):
                            g = gh * GH + gi
                            nc.vector.tensor_mul(dtx3[:, gi, :], dt6[:, g, :], xs2[:, g, :])
                        y_ps = []
                        for gi in range(GH):
                            yp = (P1 if gi == 0 else P2).tile(
                                [128, L], F32, tag="A" if gi == 0 else "B",
                                name=f"yps{bid}_{gh}_{gi}")
                            y_ps.append(yp)
                        for ns in range(NSG):
                            n0 = ns * NS
                            if gh == 0 and ns == 0 and pre_reps:
                                brep, crep = pre_reps[0]
                            else:
                                brep = prep.tile([128, NS, L], BF16, tag="brep",
                                                 name=f"brep{bid}_{gh}_{ns}")
                                nc.sync.dma_start(out=brep, in_=_bcast_rows(bc_d[n0:n0 + NS, :]))
                                crep = prep.tile([128, NS, L], BF16, tag="crep",
                                                 name=f"crep{bid}_{gh}_{ns}")
                                nc.sync.dma_start(
                                    out=crep,
                                    in_=_bcast_rows(bc_d[D_STATE + n0:D_STATE + n0 + NS, :]))
                            for gi in range(GH):
                                g = gh * GH + gi
                                slot = ns * GH + gi
                                a2 = sc.tile([128, NS, L], BF16, tag="a",
                                             name=f"a{bid}_{gh}_{ns}_{gi}")
                                for j in range(NS):
                                    nc.scalar.activation(a2[:, j, :], dt6[:, g, :], AF.Exp,
                                                         scale=An_t[:, g, n0 + j:n0 + j + 1])
                                # kill carry across state segments: a=0 at starts j>=1
                                nc.vector.memset(
                                    bass.AP(tensor=a2.tensor, offset=a2.offset + L,
                                            ap=[[a2.ap[0][0], 128], [L, NS - 1], [1, 1]]),
                                    0.0)
                                a2f = a2.rearrange("p n l -> p (n l)")
                                beng = nc.vector if (slot % 4) < b_dve else nc.gpsimd
                                b2 = sc.tile([128, NS, L], BF16, tag="b",
                                             name=f"b{bid}_{gh}_{ns}_{gi}")
                                dtx_rep = bass.AP(
                                    tensor=dtx3.tensor,
                                    offset=dtx3.offset + gi * L,
                                    ap=[[dtx3.ap[0][0], 128], [0, NS], [1, L]])
                                beng.tensor_mul(b2, dtx_rep, brep)
                                h2 = sc.tile([128, NS, L], BF16, tag="h",
                                             name=f"h{bid}_{gh}_{ns}_{gi}")
                                nc.vector.tensor_tensor_scan(
                                    h2.rearrange("p n l -> p (n l)"), a2f,
                                    b2.rearrange("p n l -> p (n l)"), 0.0, OP.mult, OP.add)
                                heng = nc.vector if ((slot + 2) % 4) < hc_dve else nc.gpsimd
                                hc2 = sc.tile([128, NS, L], BF16, tag="hc", bufs=1,
                                              name=f"hc{bid}_{gh}_{ns}_{gi}")
                                heng.tensor_mul(hc2, h2, crep)
                                for j in range(NS):
                                    mm(y_ps[gi], eye_t, hc2[:, j, :],
                                       ns == 0 and j == 0, ns == NSG - 1 and j == NS - 1)
                        for gi in range(GH):
                            g = gh * GH + gi
                            y3s = yp3.tile([128, L], BF16, tag=f"y3s{gi}",
                                           name=f"y3s{bid}_{gh}_{gi}")
                            nc.scalar.copy(y3s, y_ps[gi])
                            szg = stg.tile([128, L], BF16, tag="szg",
                                           name=f"szg{bid}_{gh}_{gi}")
                            nc.sync.dma_start(out=szg, in_=sz_d[128 * g:128 * (g + 1), :])
                            xsd = stg.tile([128, L], BF16, tag="xsd", bufs=1,
                                           name=f"xsd{bid}_{gh}_{gi}")
                            nc.vector.tensor_scalar(xsd, xs2[:, g, :],
                                                    dp_t[:, g:g + 1], None, OP.mult)
                            tmp = stg.tile([128, L], BF16, tag="gt", bufs=1,
                                           name=f"gt{bid}_{gh}_{gi}")
                            getattr(nc, gate_eng).tensor_add(tmp, xsd, y3s)
                            nc.vector.tensor_mul(ygt[:, g, :], tmp, szg)
                    for gh in (range(NG // GH) if not skip_scan and not v5 else []):
                        dtx3 = yp3.tile([128, GH, L], BF16, tag="dtx3",
                                        name=f"dtx3_{bid}_{gh}")
                        y3 = yp3.tile([128, GH, L], BF16, tag="y3",
                                      name=f"y3_{bid}_{gh}")
                        for gi in range(GH):
                            g = gh * GH + gi
                            nc.vector.tensor_mul(dtx3[:, gi, :], dt6[:, g, :], xs2[:, g, :])
                        y_ps = []
                        for gi in range(GH):
                            yp = (P1 if gi == 0 else P2).tile(
                                [128, L], F32, tag="A" if gi == 0 else "B",
                                name=f"yps{bid}_{gh}_{gi}")
                            y_ps.append(yp)
                        for n in range(D_STATE):
                            if gh == 0 and n < PREF:
                                brep, crep = pre_reps[n]
                            else:
                                brep = prep.tile([128, L], BF16, tag="brep",
                                                 name=f"brep{bid}_{gh}_{n}")
                                nc.sync.dma_start(out=brep, in_=_bcast_row(bc_d[n:n + 1, :]))
                                crep = prep.tile([128, L], BF16, tag="crep",
                                                 name=f"crep{bid}_{gh}_{n}")
                                creng = nc.scalar if dma_spread else nc.sync
                                creng.dma_start(out=crep, in_=_bcast_row(bc_d[D_STATE + n:D_STATE + n + 1, :]))
                            abh = []
                            for gi in range(GH):
                                g = gh * GH + gi
                                a = sc.tile([128, L], BF16, tag="a",
                                            name=f"a{bid}_{gh}_{n}_{gi}")
                                nc.scalar.activation(a, dt6[:, g, :], AF.Exp,
                                                     scale=An_t[:, g, n:n + 1])
                                abh.append([a, None])
                            for gi in range(GH):
                                slot = n * GH + gi
                                beng = nc.vector if (slot % 4) < b_dve else nc.gpsimd
                                b = sc.tile([128, L], BF16, tag="b",
                                            name=f"b{bid}_{gh}_{n}_{gi}")
                                beng.tensor_mul(b, dtx3[:, gi, :], brep)
                                abh[gi][1] = b
                            hs = []
                            for gi in range(GH):
                                h = sc.tile([128, L], BF16, tag="h",
                                            name=f"h{bid}_{gh}_{n}_{gi}")
                                nc.vector.tensor_tensor_scan(
                                    h, abh[gi][0], abh[gi][1], 0.0, OP.mult, OP.add)
                                hs.append(h)
                            for gi in range(GH):
                                slot = n * GH + gi
                                heng = nc.vector if ((slot + 2) % 4) < hc_dve else nc.gpsimd
                                hc = sc.tile([128, L], BF16, tag="hc",
                                             name=f"hc{bid}_{gh}_{n}_{gi}")
                                heng.tensor_mul(hc, hs[gi], crep)
                                # y += hc via identity matmul (PSUM accumulate)
                                mm(y_ps[gi], eye_t, hc, n == 0, n == D_STATE - 1)
                        for gi in range(GH):
                            g = gh * GH + gi
                            y3s = yp3.tile([128, L], BF16, tag=f"y3s{gi}",
                                           name=f"y3s{bid}_{gh}_{gi}")
                            nc.scalar.copy(y3s, y_ps[gi])
                            if debug and blk == 0:
                                nc.gpsimd.dma_start(out=dbg["dbg_y"][128 * g:128 * (g + 1), :],
                                                    in_=y3s)
                            # gate: ygt = (xs2*dp' + y) * sz2
                            if sz_sbuf:
                                szg = szp[:, g, :]
                            else:
                                szg = stg.tile([128, L], BF16, tag="szg",
                                               name=f"szg{bid}_{gh}_{gi}")
                                szeng = nc.gpsimd if (dma_spread and not sz_sp) else nc.sync
                                szeng.dma_start(out=szg, in_=sz_d[128 * g:128 * (g + 1), :])
                            xsd = stg.tile([128, L], BF16, tag="xsd",
                                           name=f"xsd{bid}_{gh}_{gi}")
                            if xsd_act:
                                nc.scalar.activation(xsd, xs2[:, g, :], AF.Identity,
                                                     scale=dp_t[:, g:g + 1])
                            else:
                                nc.vector.tensor_scalar(xsd, xs2[:, g, :],
                                                        dp_t[:, g:g + 1], None, OP.mult)
                            tmp = stg.tile([128, L], BF16, tag="gt",
                                           name=f"gt{bid}_{gh}_{gi}")
                            getattr(nc, gate_eng).tensor_add(tmp, xsd, y3s)
                            nc.vector.tensor_mul(ygt[:, g, :], tmp, szg)

                # -------- Phase C: out_proj + residual + reversed store ------
                if skip_c:
                    continue
                with ExitStack() as cctx:
                    cp = cctx.enter_context(tc.tile_pool(name=f"cp{bid}", bufs=2))
                    rin2 = cctx.enter_context(tc.tile_pool(name=f"rin2{bid}", bufs=1)) \
                        .tile([128, NM, L], F32)
                    (nc.scalar if dma_spread else nc.sync).dma_start(
                        out=rin2, in_=block_in.rearrange("(i p) t -> p i t", p=128))
                    reng = getattr(nc, resid_eng)
                    pi = 0
                    for dm in range(NM):
                        ho = cp.tile([128, L], F32, tag="ho")
                        for c in range(0, L, MMF):
                            pso = (P1 if pi % 2 == 0 else P2).tile(
                                [128, MMF], F32, tag="A" if pi % 2 == 0 else "B",
                                name=f"pso{bid}_{dm}_{c}")
                            pi += 1
                            for g in range(NG):
                                nc.tensor.matmul(pso, wo_t[:, g, 128 * dm:128 * (dm + 1)],
                                                 ygt[:, g, c:c + MMF],
                                                 start=(g == 0), stop=(g == NG - 1))
                            reng.tensor_add(ho[:, c:c + MMF], pso, rin2[:, dm, c:c + MMF])
                        hr = cp.tile([128, L], F32, tag="hr")
                        nc.vector.tensor_copy(hr, ho[:, ::-1])
                        nc.sync.dma_start(out=block_out[128 * dm:128 * (dm + 1), :], in_=hr)
                        if blk == 0:
                            hrb = cp.tile([128, L], BF16, tag="hrb")
                            nc.vector.tensor_copy(hrb, hr)
                            nc.sync.dma_start(out=h1rb_d[128 * dm:128 * (dm + 1), :], in_=hrb)

    nc.compile()
    return nc


_NC_CACHE = {}


def _get_nc(debug=False):
    if debug not in _NC_CACHE:
        _NC_CACHE[debug] = build_module(debug)
    return _NC_CACHE[debug]


def prep_host(inputs):
    """Host-side weight prep shared by all cores.

    Folds: silu computed as x*(tanh(x/2)+1) = 2*silu(x) on-device, so
      - x_proj rows all get x0.5 (xs2 = 2*silu_true); C rows get another
        x0.5 (the scan's y is 2x true because dtx2 = 2*dtx_true)
      - Dp gets x0.5 (skip term uses xs2)
      - out_proj gets x0.5 (gate uses sz2 = 2*silu_true(z))
    LN folded into in_proj (W' = w_in^T * nw plus [-S; c] extra rows).
    Conv weights become per-tap diagonal matrices for the PE.
    """
    import ml_dtypes
    f = np.float32
    bf = ml_dtypes.bfloat16
    cw = np.ascontiguousarray(inputs["conv_w"][:, :, 0, :]).astype(f)  # (2,768,4)
    diag = np.zeros((2, 128, K_CONV, NG, 128), f)
    for g in range(NG):
        blkw = cw[:, g * 128:(g + 1) * 128, :]          # (2,128,4)
        idx = np.arange(128)
        diag[:, idx, :, g, idx] = np.transpose(blkw, (1, 0, 2))  # (128,2,4)
    xp = np.ascontiguousarray(np.transpose(inputs["x_proj"], (0, 2, 1))).astype(f)
    xp = xp * 0.5
    xp[:, :, DT_RANK + D_STATE:] *= 0.5                 # C columns: x0.25 total
    wiT = np.ascontiguousarray(np.transpose(inputs["in_proj"], (0, 2, 1))).astype(f)
    wiT = wiT * inputs["norm_w"].astype(f)[:, :, None]
    s_in = wiT.sum(axis=1)                              # (2, 1536)
    wiT_raw = np.ascontiguousarray(np.transpose(inputs["in_proj"], (0, 2, 1))).astype(f)
    c_all = np.einsum('bm,bmf->bf', inputs["norm_b"].astype(f), wiT_raw)
    w4 = np.stack([-s_in, c_all], axis=1)               # (2, 2, 1536)
    return {
        "eye": np.eye(128, dtype=f).astype(bf),
        "w_in": wiT.astype(bf),
        "w4": w4.astype(bf),
        "w_out": (np.ascontiguousarray(np.transpose(inputs["out_proj"], (0, 2, 1))) * 0.5).astype(bf),
        "w_xp": xp.astype(bf),
        "w_dt": np.ascontiguousarray(np.transpose(inputs["dt_w"], (0, 2, 1))).astype(bf),
        "cwd": diag.astype(bf),
        "cbh": (0.5 * inputs["conv_b"]).astype(f),
        "dtb": inputs["dt_b"].astype(f),
        "An": (-np.exp(inputs["A_log"])).astype(f),
        "dp": (0.5 * inputs["Dp"]).astype(f),
    }


def build_module_repeat(k):
    return build_module(False, repeat=k)


def kernel(**inputs):
    inputs = {k: np.asarray(v) for k, v in inputs.items()}
    nc = _get_nc(False)
    shared = prep_host(inputs)
    import ml_dtypes
    in_maps = []
    for s in range(BATCH):
        m = dict(shared)
        xt = np.ascontiguousarray(inputs["x"][s].T).astype(np.float32)
        m["xT"] = xt
        m["xTb"] = xt.astype(ml_dtypes.bfloat16)
        in_maps.append(m)
    res = run_bass_kernel_spmd(nc, in_maps, list(range(BATCH)))
    out = np.stack([res.results[s]["outT"].T for s in range(BATCH)])
    return np.ascontiguousarray(out.astype(np.float32))



# revision 49
# speedup vs baseline: 1.5758x; 1.1308x over previous
"""Bi-Mamba Trainium2 kernel (v4.1 — HW-calibrated engine rebalance).

Changes vs v4 baseline, driven by slope-microbenchmark HW calibration:
  - Pool/gpsimd tensor_tensor muls measure ~4.0us per [128,2048] bf16 tile
    (no bf16 packing on the Q7 cores; ~4x slower than DVE's 1.0us) and DVE
    tensor_tensor_scan measures 2 cyc/elem (~4.3us, dtype-independent), so
    the v4 Pool-heavy scan assignment made Pool the bottleneck. The b/hc
    broadcast muls are rebalanced (b_dve=2, hc_dve=1 of 4 slots to DVE)
    and the gate add moved to DVE.
  - A single [1->128]-partition broadcast DMA measures ~4.0us (~130GB/s per
    hardware queue); the scan issues 96 of them per block plus sz/weight/IO
    traffic, so a single SP queue (~67MB/block) was co-limiting. DMAs are
    now spread across queues: C-row broadcasts issue from the ACT hwdge
    queue, sz gate loads stay on SP, phase-C residual loads on ACT.
    (Routing sz loads through gpsimd SWDGE measured worse - descriptor
    generation steals Pool engine time; extra ACT compute ops also stall
    ACT-queue DMA issue, so ACT carries DMAs but no extra compute.)
Round-interleaved HW A/B: spread+rebalance ~0.9-1.2ms vs ~1.5-1.9ms for
rebalance-only vs ~2.6-2.9ms for v4, same-session units. An NS=2
state-blocked scan variant (v5, cfg-gated) validated numerically but
measured slower on HW (stride-0 replicated operands defeat DVE 2x packing).


Contract: kernel(**inputs) takes the FULL unsharded inputs (numpy) keyed as
reference.setup_inputs() and returns the FULL (8, 2048, 384) float32 output.

Sharding: pure data-parallel over batch — 8 samples, 8 cores, one sample per
core, no collectives. All weights are replicated per core.

Design notes (per core, channel-major [feature, token] layout):
  - LayerNorm is folded into the in_proj matmul: stats via PE ones-matmuls,
    rsqrt via bit-trick+Newton on token-major-reshaped 16-wide tiles, then
    the matmul input is x*rs and two extra contraction rows [mu*rs; 1] with
    host-prepared weights [-S[f]; c[f]] complete the normalized projection.
  - silu(x) is computed as x*(tanh(x/2)+1) = 2*silu(x) with the 0.5 folded
    into host-prepped weights (x_proj rows, Dp, out_proj): ACT tanh + one
    4x-mode tensor_scalar + one 2x-mode bf16 mul, no 1x-mode STT ops.
  - causal depthwise conv (k=4) runs on the PE as 4 diagonal-matrix matmuls
    accumulated in PSUM (diagonals built on the host).
  - everything that streams is bf16 (DVE TensorTensor is 2x only for 2-byte
    packed dtypes; matmuls are full rate in bf16).
  - scan: per (128-channel group, state): a = exp(A*dt) on ACT, b = dtx*B_rep
    (DVE/Pool), h = tensor_tensor_scan(a,b) [DVE-only, 1x], hc = h*C_rep
    (DVE/Pool), and y += hc happens on the PE as an identity-matmul PSUM
    accumulation (GH=2 groups in flight, 2x4 PSUM banks).
  - out_proj accumulates over the 6 channel groups in PSUM, then residual-add
    and a reversed store so the second block runs identical code on the
    flipped sequence.
  - two shared block-scope PSUM pools and a single scan scope avoid
    pool-release barriers inside the scan; the first broadcast DMAs are
    prefetched before the phase-A pool release barrier.
"""
import numpy as np
from contextlib import ExitStack

import concourse.bass as bass
import concourse.tile as tile
from concourse import bacc, mybir
from concourse.bass_utils import run_bass_kernel_spmd

F32 = mybir.dt.float32
BF16 = mybir.dt.bfloat16
AF = mybir.ActivationFunctionType
OP = mybir.AluOpType

D_MODEL = 384
D_INNER = 768
D_STATE = 16
DT_RANK = 24
K_CONV = 4
L = 2048
BATCH = 8
EPS = 1e-5
NG = D_INNER // 128      # 6 channel blocks of d_inner
NM = D_MODEL // 128      # 3 channel blocks of d_model


def _bcast_row(ap):
    """View a [1, L] AP as [128, L] with partition step 0 (replicated read)."""
    return bass.AP(tensor=ap.tensor, offset=ap.offset, ap=[[0, 128]] + list(ap.ap[1:]))


def _bcast_rows(ap):
    """View an [R, L] DRAM AP as [128, R, L] with partition step 0."""
    return bass.AP(tensor=ap.tensor, offset=ap.offset, ap=[[0, 128]] + list(ap.ap))


def build_module(debug=False, repeat=1, cfg=None):
    cfg = cfg or {}
    b_dve = cfg.get('b_dve', 2)        # of 4 slots, how many b-muls go to DVE
    hc_dve = cfg.get('hc_dve', 1)
    gate_eng = cfg.get('gate_eng', 'vector')
    resid_eng = cfg.get('resid_eng', 'vector')
    GH = 2
    sc_bufs = cfg.get('sc_bufs', 2)
    PREF = cfg.get('pref', 2)          # broadcast loads prefetched pre-barrier
    skip_scan = cfg.get('skip_scan', False)   # timing ablation only
    skip_c = cfg.get('skip_c', False)         # timing ablation only
    v5 = cfg.get('v5', False)                 # state-blocked scan (slower on HW)
    NS = cfg.get('ns', 2)                     # states per tts op (v5)
    xsd_act = cfg.get('xsd_act', False)       # xsd = xs2*dp' on ACT instead of DVE
    dma_spread = cfg.get('dma_spread', True)   # crep via ACT queue, szg via SWDGE
    sz_sbuf = cfg.get('sz_sbuf', False)        # keep sz2 in SBUF (no DRAM roundtrip)
    sz_sp = cfg.get('sz_sp', True)             # with dma_spread: szg stays on SP
    szst_act = cfg.get('szst_act', False)      # sz stores via ACT queue
    NCACHE = cfg.get('ncache', 4)              # B/C rows cached across gh passes
    brep_alt = cfg.get('brep_alt', False)      # alternate uncached brep SP/ACT
    szg_act = cfg.get('szg_act', False)        # gate sz loads via ACT queue

    nc = bacc.Bacc("TRN2", target_bir_lowering=False, debug=False)

    def din(name, shape, dt=F32):
        return nc.dram_tensor(name, shape, dt, kind="ExternalInput").ap()

    xT = din("xT", [D_MODEL, L])
    xTb = din("xTb", [D_MODEL, L], BF16)
    eye = din("eye", [128, 128], BF16)
    w_in = din("w_in", [2, D_MODEL, 2 * D_INNER], BF16)   # host: folded x norm_w
    w4 = din("w4", [2, 2, 2 * D_INNER], BF16)             # [-S[f]; c[f]] rows
    w_out = din("w_out", [2, D_INNER, D_MODEL], BF16)     # host-scaled x0.5
    w_xp = din("w_xp", [2, D_INNER, 56], BF16)            # host-scaled (see prep)
    w_dt = din("w_dt", [2, DT_RANK, D_INNER], BF16)
    cwd = din("cwd", [2, 128, K_CONV, NG, 128], BF16)     # host-built diagonals
    cbh = din("cbh", [2, D_INNER])                        # 0.5*conv_b
    dtb = din("dtb", [2, D_INNER])
    An = din("An", [2, D_INNER, D_STATE])                 # -exp(A_log)
    dp = din("dp", [2, D_INNER])                          # host-scaled x0.5
    outT = nc.dram_tensor("outT", [D_MODEL, L], F32, kind="ExternalOutput").ap()

    bc_d = nc.dram_tensor("bc_d", [2 * D_STATE, L], BF16).ap()
    sz_d = nc.dram_tensor("sz_d", [D_INNER, L], BF16).ap()
    h1r_d = nc.dram_tensor("h1r_d", [D_MODEL, L], F32).ap()
    h1rb_d = nc.dram_tensor("h1rb_d", [D_MODEL, L], BF16).ap()

    dbg = {}
    if debug:
        for name, shape in [("dbg_xs", [D_INNER, L]),
                            ("dbg_dt", [D_INNER, L]), ("dbg_y", [D_INNER, L]),
                            ("dbg_xdbl", [56, L])]:
            dbg[name] = nc.dram_tensor(name, shape, F32, kind="ExternalOutput").ap()

    MMF = 512  # max free-dim columns per matmul (one PSUM bank of fp32)

    def mm(out, lhsT, rhs, first, last):
        """k-accumulating matmul, split into 512-column chunks."""
        F = rhs.shape[-1]
        for c in range(0, F, MMF):
            nc.tensor.matmul(out[:, c:c + MMF], lhsT, rhs[:, c:c + MMF],
                             start=first, stop=last)

    with tile.TileContext(nc) as tc, ExitStack() as ctx:
        consts = ctx.enter_context(tc.tile_pool(name="consts", bufs=1))
        ones_col = consts.tile([128, 1], BF16)
        nc.vector.memset(ones_col, 1.0)
        eye_t = consts.tile([128, 128], BF16)
        nc.sync.dma_start(out=eye_t, in_=eye)
        c_eps = consts.tile([1, 1], F32)
        nc.vector.memset(c_eps, EPS)

        for bid, blk in [(r * 2 + b, b) for r in range(repeat) for b in range(2)]:
            block_in = xT if blk == 0 else h1r_d
            block_in_bf = xTb if blk == 0 else h1rb_d
            block_out = h1r_d if blk == 0 else outT
            with ExitStack() as bctx:
                # ---- block-lifetime pools (stack order matters) ----
                wp = bctx.enter_context(tc.tile_pool(name=f"w{bid}", bufs=1))
                pers = bctx.enter_context(tc.tile_pool(name=f"pers{bid}", bufs=1))
                xs2 = pers.tile([128, NG, L], BF16)    # 2*silu(conv)
                ygt = pers.tile([128, NG, L], BF16)    # gated output
                x_dbl = pers.tile([56, L], BF16)
                szp = pers.tile([128, NG, L], BF16, name=f"szp{bid}") if sz_sbuf else None
                bp = bctx.enter_context(tc.tile_pool(name=f"bp{bid}", bufs=1))
                dt6 = bp.tile([128, NG, L], BF16, tag="dt6")
                prep = bctx.enter_context(tc.tile_pool(name=f"prep{bid}", bufs=2))
                P1 = bctx.enter_context(tc.tile_pool(name=f"P1_{bid}", bufs=1, space="PSUM"))
                P2 = bctx.enter_context(tc.tile_pool(name=f"P2_{bid}", bufs=1, space="PSUM"))

                # ---- phase A scope ----
                actx = bctx.enter_context(ExitStack())
                ap_big = actx.enter_context(tc.tile_pool(name=f"abig{bid}", bufs=1))
                ap_row = actx.enter_context(tc.tile_pool(name=f"arow{bid}", bufs=1))
                ap_rep = actx.enter_context(tc.tile_pool(name=f"arep{bid}", bufs=1))
                ap_sm = actx.enter_context(tc.tile_pool(name=f"asm{bid}", bufs=2))

                # input first: everything hangs off rin, so its DMA must not
                # queue behind the weight loads
                rin = ap_big.tile([128, NM, L], BF16)
                for g in range(NM):
                    nc.sync.dma_start(out=rin[:, g, :],
                                      in_=block_in_bf[128 * g:128 * (g + 1), :])

                wo_t = wp.tile([128, NG, D_MODEL], BF16)
                nc.sync.dma_start(out=wo_t, in_=w_out[blk].rearrange("(k p) m -> p k m", p=128))
                wdt_t = wp.tile([DT_RANK, D_INNER], BF16)
                nc.sync.dma_start(out=wdt_t, in_=w_dt[blk])
                w4_t = wp.tile([2, 2 * D_INNER], BF16)
                nc.sync.dma_start(out=w4_t, in_=w4[blk])
                cbh_t = wp.tile([128, NG], F32)
                nc.sync.dma_start(out=cbh_t, in_=cbh[blk].rearrange("(g p) -> p g", p=128))
                dtb_t = wp.tile([128, NG], F32)
                nc.sync.dma_start(out=dtb_t, in_=dtb[blk].rearrange("(g p) -> p g", p=128))
                dp_t = wp.tile([128, NG], F32)
                nc.sync.dma_start(out=dp_t, in_=dp[blk].rearrange("(g p) -> p g", p=128))
                An_t = wp.tile([128, NG, D_STATE], F32)
                nc.sync.dma_start(out=An_t, in_=An[blk].rearrange("(g p) n -> p g n", p=128))
                wi_t = ap_big.tile([128, NM, 2 * D_INNER], BF16)
                nc.sync.dma_start(out=wi_t, in_=w_in[blk].rearrange("(k p) m -> p k m", p=128))
                wxp_t = ap_big.tile([128, NG, 56], BF16)
                nc.sync.dma_start(out=wxp_t, in_=w_xp[blk].rearrange("(k p) m -> p k m", p=128))
                diag_t = ap_big.tile([128, K_CONV, NG, 128], BF16)
                nc.sync.dma_start(out=diag_t, in_=cwd[blk])

                # LN stats: sum and sum-of-squares rows via ones-matmul
                mu_ps = P1.tile([1, L], F32, tag="A")
                for g in range(NM):
                    mm(mu_ps, ones_col, rin[:, g, :], g == 0, g == NM - 1)
                sq_ps = P2.tile([1, L], F32, tag="B")
                for g in range(NM):
                    sq = ap_sm.tile([128, L], BF16, tag="t1")
                    nc.vector.tensor_mul(sq, rin[:, g, :], rin[:, g, :])
                    mm(sq_ps, ones_col, sq, g == 0, g == NM - 1)
                rows2 = ap_row.tile([1, 2, L], BF16)
                mu_row = rows2[:, 0, :]
                var_row = rows2[:, 1, :]
                nc.scalar.activation(mu_row, mu_ps, AF.Identity, scale=1.0 / D_MODEL)
                nc.scalar.activation(var_row, sq_ps, AF.Identity, scale=1.0 / D_MODEL,
                                     bias=c_eps)
                # token-major reshape (t = 16p + c; any bijection works since
                # the rsqrt is elementwise) so Newton runs on 16-wide tiles
                LT16 = L // 128
                tmb2 = ap_row.tile([128, 2, LT16], BF16)
                for r in range(2):
                    nc.sync.dma_start(
                        out=tmb2[:, r, :],
                        in_=bass.AP(tensor=rows2.tensor, offset=rows2.offset + r * L,
                                    ap=[[rows2.ap[0][0], 1], [LT16, 128], [1, LT16]]))
                tm = ap_row.tile([128, 4, LT16], F32)
                mu_tm = tm[:, 0, :]
                var_tm = tm[:, 1, :]
                yr = tm[:, 2, :]
                tnw = tm[:, 3, :]
                nc.vector.tensor_copy(mu_tm, tmb2[:, 0, :])
                nc.vector.tensor_copy(var_tm, tmb2[:, 1, :])
                nc.vector.tensor_mul(tnw, mu_tm, mu_tm)
                nc.vector.tensor_sub(var_tm, var_tm, tnw)
                # rstd = rsqrt(var+eps) via bit-trick seed + 2 Newton steps
                I32 = mybir.dt.int32
                yi = yr.bitcast(I32)
                nc.vector.tensor_scalar(yi, var_tm.bitcast(I32), 1, None,
                                        OP.arith_shift_right)
                nc.vector.tensor_scalar(yi, yi, -1, 0x5f3759df, OP.mult, OP.add)
                for _ in range(2):
                    nc.vector.tensor_mul(tnw, yr, yr)
                    nc.vector.tensor_mul(tnw, tnw, var_tm)
                    nc.vector.tensor_scalar(tnw, tnw, -0.5, 1.5, OP.mult, OP.add)
                    nc.vector.tensor_mul(yr, yr, tnw)
                # rs -> row (reuse mu slot) for broadcast; [mu*rs; 1] -> ex2
                tmb = ap_row.tile([128, 2, LT16], BF16)
                nc.vector.tensor_copy(tmb[:, 0, :], yr)
                nc.vector.tensor_mul(mu_tm, mu_tm, yr)
                nc.vector.tensor_copy(tmb[:, 1, :], mu_tm)
                ex2 = ap_row.tile([2, L], BF16)
                nc.vector.memset(ex2, 1.0)
                nc.sync.dma_start(
                    out=bass.AP(tensor=rows2.tensor, offset=rows2.offset,
                                ap=[[rows2.ap[0][0], 1], [LT16, 128], [1, LT16]]),
                    in_=tmb[:, 0, :])
                nc.sync.dma_start(
                    out=bass.AP(tensor=ex2.tensor, offset=ex2.offset,
                                ap=[[ex2.ap[0][0], 1], [LT16, 128], [1, LT16]]),
                    in_=tmb[:, 1, :])
                rs_rep = ap_rep.tile([128, L], BF16, tag="rep_rs")
                nc.gpsimd.partition_broadcast(rs_rep, rows2[:, 0, :])
                for g in range(NM):
                    nc.vector.tensor_mul(rin[:, g, :], rin[:, g, :], rs_rep)

                # in_proj: 12 output feature tiles, LN fully folded in
                xsp = ap_big.tile([128, NG, K_CONV - 1 + L], BF16)
                nc.vector.memset(xsp[:, :, 0:K_CONV - 1], 0.0)
                for f in range(2 * NG):
                    ps = (P1 if f % 2 == 0 else P2).tile(
                        [128, L], F32, tag="A" if f % 2 == 0 else "B")
                    for k in range(NM):
                        mm(ps, wi_t[:, k, 128 * f:128 * (f + 1)], rin[:, k, :],
                           k == 0, False)
                    mm(ps, w4_t[:, 128 * f:128 * (f + 1)], ex2, False, True)
                    if f < NG:
                        nc.scalar.copy(xsp[:, f, K_CONV - 1:], ps)
                    else:
                        g = f - NG
                        zt = ap_sm.tile([128, L], BF16, tag="t2")
                        nc.scalar.copy(zt, ps)
                        tz = ap_sm.tile([128, L], BF16, tag="t3")
                        nc.scalar.activation(tz, zt, AF.Tanh, scale=0.5)
                        uz = ap_sm.tile([128, L], BF16, tag="t1")
                        nc.vector.tensor_scalar_add(uz, tz, 1.0)
                        if sz_sbuf:
                            nc.vector.tensor_mul(szp[:, g, :], uz, zt)
                        else:
                            szt = ap_sm.tile([128, L], BF16, tag="t4")
                            nc.vector.tensor_mul(szt, uz, zt)
                            (nc.scalar if szst_act else nc.sync).dma_start(
                                out=sz_d[128 * g:128 * (g + 1), :], in_=szt)

                # conv on PE (diag matmuls, P2) + tanh-silu + x_proj accum (P1)
                px = P1.tile([56, L], F32, tag="A")
                for g in range(NG):
                    ps2 = P2.tile([128, L], F32, tag="B")
                    for j in range(K_CONV):
                        mm(ps2, diag_t[:, j, g, :], xsp[:, g, j:j + L],
                           j == 0, j == K_CONV - 1)
                    cfh = ap_sm.tile([128, L], BF16, tag="t2")
                    nc.scalar.activation(cfh, ps2, AF.Identity, scale=0.5,
                                         bias=cbh_t[:, g:g + 1])
                    tc_t = ap_sm.tile([128, L], BF16, tag="t3")
                    nc.scalar.activation(tc_t, cfh, AF.Tanh)
                    u2 = ap_sm.tile([128, L], BF16, tag="t1")
                    nc.vector.tensor_scalar(u2, tc_t, 2.0, 2.0, OP.mult, OP.add)
                    nc.vector.tensor_mul(xs2[:, g, :], u2, cfh)
                    mm(px, wxp_t[:, g, :], xs2[:, g, :], g == 0, g == NG - 1)
                    if debug and blk == 0:
                        nc.gpsimd.dma_start(out=dbg["dbg_xs"][128 * g:128 * (g + 1), :], in_=xs2[:, g, :])
                nc.scalar.copy(x_dbl, px)
                nc.sync.dma_start(out=bc_d, in_=x_dbl[DT_RANK:DT_RANK + 2 * D_STATE, :])
                if debug and blk == 0:
                    nc.gpsimd.dma_start(out=dbg["dbg_xdbl"], in_=x_dbl)

                # dt for all groups (softplus(z) = u*(1-u/2), u=e^z, z<=-3.5)
                for g in range(NG):
                    psd = (P1 if g % 2 == 0 else P2).tile(
                        [128, L], F32, tag="A" if g % 2 == 0 else "B")
                    mm(psd, wdt_t[:, 128 * g:128 * (g + 1)], x_dbl[0:DT_RANK, :],
                       True, True)
                    uu = ap_sm.tile([128, L], BF16, tag="t2")
                    nc.scalar.activation(uu, psd, AF.Exp, bias=dtb_t[:, g:g + 1])
                    t0 = ap_sm.tile([128, L], BF16, tag="t3")
                    nc.vector.tensor_scalar(t0, uu, -0.5, 1.0, OP.mult, OP.add)
                    nc.vector.tensor_mul(dt6[:, g, :], t0, uu)
                    if debug and blk == 0:
                        nc.gpsimd.dma_start(out=dbg["dbg_dt"][128 * g:128 * (g + 1), :], in_=dt6[:, g, :])

                # prefetch first broadcast loads before the pool-release barrier
                pre_reps = []
                if skip_scan or NCACHE > 0:
                    PREF = 0
                if v5:
                    pass   # broadcasts allocated inside the scan scope
                else:
                    for n in range(PREF):
                        brep = prep.tile([128, L], BF16, tag="brep")
                        nc.sync.dma_start(out=brep, in_=_bcast_row(bc_d[n:n + 1, :]))
                        crep = prep.tile([128, L], BF16, tag="crep")
                        nc.sync.dma_start(out=crep, in_=_bcast_row(bc_d[D_STATE + n:D_STATE + n + 1, :]))
                        pre_reps.append((brep, crep))

                actx.close()   # release phase-A pools (one barrier)

                # ---------------- Phase B: scan ------------------------------
                if skip_scan:
                    nc.vector.memset(ygt, 0.0)
                with ExitStack() as sctx:
                    yp3 = sctx.enter_context(tc.tile_pool(name=f"yp{bid}", bufs=1))
                    stg = sctx.enter_context(tc.tile_pool(name=f"stg{bid}", bufs=2))
                    sc = sctx.enter_context(tc.tile_pool(name=f"sc{bid}", bufs=sc_bufs))
                    if v5:
                        prep = sctx.enter_context(tc.tile_pool(name=f"prep2_{bid}", bufs=2))
                    if NCACHE > 0:
                        bcp = sctx.enter_context(tc.tile_pool(name=f"bcp{bid}", bufs=1))
                    cacheB, cacheC = {}, {}
                    NSG = D_STATE // NS
                    for gh in (range(NG // GH) if (v5 and not skip_scan) else []):
                        # ---- v5: state-blocked scan (NS states per op) ----
                        dtx3 = yp3.tile([128, GH, L], BF16, tag="dtx3",
                                        name=f"dtx3_{bid}_{gh}")
                        for gi in range(GH):
                            g = gh * GH + gi
                            nc.vector.tensor_mul(dtx3[:, gi, :], dt6[:, g, :], xs2[:, g, :])
                        y_ps = []
                        for gi in range(GH):
                            yp = (P1 if gi == 0 else P2).tile(
                                [128, L], F32, tag="A" if gi == 0 else "B",
                                name=f"yps{bid}_{gh}_{gi}")
                            y_ps.append(yp)
                        for ns in range(NSG):
                            n0 = ns * NS
                            if gh == 0 and ns == 0 and pre_reps:
                                brep, crep = pre_reps[0]
                            else:
                                brep = prep.tile([128, NS, L], BF16, tag="brep",
                                                 name=f"brep{bid}_{gh}_{ns}")
                                nc.sync.dma_start(out=brep, in_=_bcast_rows(bc_d[n0:n0 + NS, :]))
                                crep = prep.tile([128, NS, L], BF16, tag="crep",
                                                 name=f"crep{bid}_{gh}_{ns}")
                                nc.sync.dma_start(
                                    out=crep,
                                    in_=_bcast_rows(bc_d[D_STATE + n0:D_STATE + n0 + NS, :]))
                            for gi in range(GH):
                                g = gh * GH + gi
                                slot = ns * GH + gi
                                a2 = sc.tile([128, NS, L], BF16, tag="a",
                                             name=f"a{bid}_{gh}_{ns}_{gi}")
                                for j in range(NS):
                                    nc.scalar.activation(a2[:, j, :], dt6[:, g, :], AF.Exp,
                                                         scale=An_t[:, g, n0 + j:n0 + j + 1])
                                # kill carry across state segments: a=0 at starts j>=1
                                nc.vector.memset(
                                    bass.AP(tensor=a2.tensor, offset=a2.offset + L,
                                            ap=[[a2.ap[0][0], 128], [L, NS - 1], [1, 1]]),
                                    0.0)
                                a2f = a2.rearrange("p n l -> p (n l)")
                                beng = nc.vector if (slot % 4) < b_dve else nc.gpsimd
                                b2 = sc.tile([128, NS, L], BF16, tag="b",
                                             name=f"b{bid}_{gh}_{ns}_{gi}")
                                dtx_rep = bass.AP(
                                    tensor=dtx3.tensor,
                                    offset=dtx3.offset + gi * L,
                                    ap=[[dtx3.ap[0][0], 128], [0, NS], [1, L]])
                                beng.tensor_mul(b2, dtx_rep, brep)
                                h2 = sc.tile([128, NS, L], BF16, tag="h",
                                             name=f"h{bid}_{gh}_{ns}_{gi}")
                                nc.vector.tensor_tensor_scan(
                                    h2.rearrange("p n l -> p (n l)"), a2f,
                                    b2.rearrange("p n l -> p (n l)"), 0.0, OP.mult, OP.add)
                                heng = nc.vector if ((slot + 2) % 4) < hc_dve else nc.gpsimd
                                hc2 = sc.tile([128, NS, L], BF16, tag="hc", bufs=1,
                                              name=f"hc{bid}_{gh}_{ns}_{gi}")
                                heng.tensor_mul(hc2, h2, crep)
                                for j in range(NS):
                                    mm(y_ps[gi], eye_t, hc2[:, j, :],
                                       ns == 0 and j == 0, ns == NSG - 1 and j == NS - 1)
                        for gi in range(GH):
                            g = gh * GH + gi
                            y3s = yp3.tile([128, L], BF16, tag=f"y3s{gi}",
                                           name=f"y3s{bid}_{gh}_{gi}")
                            nc.scalar.copy(y3s, y_ps[gi])
                            szg = stg.tile([128, L], BF16, tag="szg",
                                           name=f"szg{bid}_{gh}_{gi}")
                            nc.sync.dma_start(out=szg, in_=sz_d[128 * g:128 * (g + 1), :])
                            xsd = stg.tile([128, L], BF16, tag="xsd", bufs=1,
                                           name=f"xsd{bid}_{gh}_{gi}")
                            nc.vector.tensor_scalar(xsd, xs2[:, g, :],
                                                    dp_t[:, g:g + 1], None, OP.mult)
                            tmp = stg.tile([128, L], BF16, tag="gt", bufs=1,
                                           name=f"gt{bid}_{gh}_{gi}")
                            getattr(nc, gate_eng).tensor_add(tmp, xsd, y3s)
                            nc.vector.tensor_mul(ygt[:, g, :], tmp, szg)
                    for gh in (range(NG // GH) if not skip_scan and not v5 else []):
                        dtx3 = yp3.tile([128, GH, L], BF16, tag="dtx3",
                                        name=f"dtx3_{bid}_{gh}")
                        for gi in range(GH):
                            g = gh * GH + gi
                            nc.vector.tensor_mul(dtx3[:, gi, :], dt6[:, g, :], xs2[:, g, :])
                        y_ps = []
                        for gi in range(GH):
                            yp = (P1 if gi == 0 else P2).tile(
                                [128, L], F32, tag="A" if gi == 0 else "B",
                                name=f"yps{bid}_{gh}_{gi}")
                            y_ps.append(yp)
                        for n in range(D_STATE):
                            creng = nc.scalar if dma_spread else nc.sync
                            if n < NCACHE:
                                if gh == 0:
                                    cb = bcp.tile([128, L], BF16, tag=f"cb{n}",
                                                  name=f"cb{bid}_{n}")
                                    nc.sync.dma_start(out=cb, in_=_bcast_row(bc_d[n:n + 1, :]))
                                    cc = bcp.tile([128, L], BF16, tag=f"cc{n}",
                                                  name=f"cc{bid}_{n}")
                                    creng.dma_start(out=cc, in_=_bcast_row(bc_d[D_STATE + n:D_STATE + n + 1, :]))
                                    cacheB[n], cacheC[n] = cb, cc
                                brep, crep = cacheB[n], cacheC[n]
                            elif gh == 0 and n < PREF:
                                brep, crep = pre_reps[n]
                            else:
                                brep = prep.tile([128, L], BF16, tag="brep",
                                                 name=f"brep{bid}_{gh}_{n}")
                                breng = creng if (brep_alt and n % 2 == 1) else nc.sync
                                breng.dma_start(out=brep, in_=_bcast_row(bc_d[n:n + 1, :]))
                                crep = prep.tile([128, L], BF16, tag="crep",
                                                 name=f"crep{bid}_{gh}_{n}")
                                creng.dma_start(out=crep, in_=_bcast_row(bc_d[D_STATE + n:D_STATE + n + 1, :]))
                            abh = []
                            for gi in range(GH):
                                g = gh * GH + gi
                                a = sc.tile([128, L], BF16, tag="a",
                                            name=f"a{bid}_{gh}_{n}_{gi}")
                                nc.scalar.activation(a, dt6[:, g, :], AF.Exp,
                                                     scale=An_t[:, g, n:n + 1])
                                abh.append([a, None])
                            for gi in range(GH):
                                slot = n * GH + gi
                                beng = nc.vector if (slot % 4) < b_dve else nc.gpsimd
                                b = sc.tile([128, L], BF16, tag="b",
                                            name=f"b{bid}_{gh}_{n}_{gi}")
                                beng.tensor_mul(b, dtx3[:, gi, :], brep)
                                abh[gi][1] = b
                            hs = []
                            for gi in range(GH):
                                h = sc.tile([128, L], BF16, tag="h",
                                            name=f"h{bid}_{gh}_{n}_{gi}")
                                nc.vector.tensor_tensor_scan(
                                    h, abh[gi][0], abh[gi][1], 0.0, OP.mult, OP.add)
                                hs.append(h)
                            for gi in range(GH):
                                slot = n * GH + gi
                                heng = nc.vector if ((slot + 2) % 4) < hc_dve else nc.gpsimd
                                hc = sc.tile([128, L], BF16, tag="hc",
                                             name=f"hc{bid}_{gh}_{n}_{gi}")
                                heng.tensor_mul(hc, hs[gi], crep)
                                # y += hc via identity matmul (PSUM accumulate)
                                mm(y_ps[gi], eye_t, hc, n == 0, n == D_STATE - 1)
                        for gi in range(GH):
                            g = gh * GH + gi
                            y3s = yp3.tile([128, L], BF16, tag=f"y3s{gi}",
                                           name=f"y3s{bid}_{gh}_{gi}")
                            nc.scalar.copy(y3s, y_ps[gi])
                            if debug and blk == 0:
                                nc.gpsimd.dma_start(out=dbg["dbg_y"][128 * g:128 * (g + 1), :],
                                                    in_=y3s)
                            # gate: ygt = (xs2*dp' + y) * sz2
                            if sz_sbuf:
                                szg = szp[:, g, :]
                            else:
                                szg = stg.tile([128, L], BF16, tag="szg",
                                               name=f"szg{bid}_{gh}_{gi}")
                                szeng = (nc.scalar if szg_act else
                                         nc.gpsimd if (dma_spread and not sz_sp) else nc.sync)
                                szeng.dma_start(out=szg, in_=sz_d[128 * g:128 * (g + 1), :])
                            xsd = stg.tile([128, L], BF16, tag="xsd",
                                           name=f"xsd{bid}_{gh}_{gi}")
                            if xsd_act:
                                nc.scalar.activation(xsd, xs2[:, g, :], AF.Identity,
                                                     scale=dp_t[:, g:g + 1])
                            else:
                                nc.vector.tensor_scalar(xsd, xs2[:, g, :],
                                                        dp_t[:, g:g + 1], None, OP.mult)
                            tmp = stg.tile([128, L], BF16, tag="gt",
                                           name=f"gt{bid}_{gh}_{gi}")
                            getattr(nc, gate_eng).tensor_add(tmp, xsd, y3s)
                            nc.vector.tensor_mul(ygt[:, g, :], tmp, szg)

                # -------- Phase C: out_proj + residual + reversed store ------
                if skip_c:
                    continue
                with ExitStack() as cctx:
                    cp = cctx.enter_context(tc.tile_pool(name=f"cp{bid}", bufs=2))
                    rin2 = cctx.enter_context(tc.tile_pool(name=f"rin2{bid}", bufs=1)) \
                        .tile([128, NM, L], F32)
                    (nc.scalar if dma_spread else nc.sync).dma_start(
                        out=rin2, in_=block_in.rearrange("(i p) t -> p i t", p=128))
                    reng = getattr(nc, resid_eng)
                    pi = 0
                    for dm in range(NM):
                        ho = cp.tile([128, L], F32, tag="ho")
                        for c in range(0, L, MMF):
                            pso = (P1 if pi % 2 == 0 else P2).tile(
                                [128, MMF], F32, tag="A" if pi % 2 == 0 else "B",
                                name=f"pso{bid}_{dm}_{c}")
                            pi += 1
                            for g in range(NG):
                                nc.tensor.matmul(pso, wo_t[:, g, 128 * dm:128 * (dm + 1)],
                                                 ygt[:, g, c:c + MMF],
                                                 start=(g == 0), stop=(g == NG - 1))
                            reng.tensor_add(ho[:, c:c + MMF], pso, rin2[:, dm, c:c + MMF])
                        hr = cp.tile([128, L], F32, tag="hr")
                        nc.vector.tensor_copy(hr, ho[:, ::-1])
                        nc.sync.dma_start(out=block_out[128 * dm:128 * (dm + 1), :], in_=hr)
                        if blk == 0:
                            hrb = cp.tile([128, L], BF16, tag="hrb")
                            nc.vector.tensor_copy(hrb, hr)
                            nc.sync.dma_start(out=h1rb_d[128 * dm:128 * (dm + 1), :], in_=hrb)

    nc.compile()
    return nc


_NC_CACHE = {}


def _get_nc(debug=False):
    if debug not in _NC_CACHE:
        _NC_CACHE[debug] = build_module(debug)
    return _NC_CACHE[debug]


def prep_host(inputs):
    """Host-side weight prep shared by all cores.

    Folds: silu computed as x*(tanh(x/2)+1) = 2*silu(x) on-device, so
      - x_proj rows all get x0.5 (xs2 = 2*silu_true); C rows get another
        x0.5 (the scan's y is 2x true because dtx2 = 2*dtx_true)
      - Dp gets x0.5 (skip term uses xs2)
      - out_proj gets x0.5 (gate uses sz2 = 2*silu_true(z))
    LN folded into in_proj (W' = w_in^T * nw plus [-S; c] extra rows).
    Conv weights become per-tap diagonal matrices for the PE.
    """
    import ml_dtypes
    f = np.float32
    bf = ml_dtypes.bfloat16
    cw = np.ascontiguousarray(inputs["conv_w"][:, :, 0, :]).astype(f)  # (2,768,4)
    diag = np.zeros((2, 128, K_CONV, NG, 128), f)
    for g in range(NG):
        blkw = cw[:, g * 128:(g + 1) * 128, :]          # (2,128,4)
        idx = np.arange(128)
        diag[:, idx, :, g, idx] = np.transpose(blkw, (1, 0, 2))  # (128,2,4)
    xp = np.ascontiguousarray(np.transpose(inputs["x_proj"], (0, 2, 1))).astype(f)
    xp = xp * 0.5
    xp[:, :, DT_RANK + D_STATE:] *= 0.5                 # C columns: x0.25 total
    wiT = np.ascontiguousarray(np.transpose(inputs["in_proj"], (0, 2, 1))).astype(f)
    wiT = wiT * inputs["norm_w"].astype(f)[:, :, None]
    s_in = wiT.sum(axis=1)                              # (2, 1536)
    wiT_raw = np.ascontiguousarray(np.transpose(inputs["in_proj"], (0, 2, 1))).astype(f)
    c_all = np.einsum('bm,bmf->bf', inputs["norm_b"].astype(f), wiT_raw)
    w4 = np.stack([-s_in, c_all], axis=1)               # (2, 2, 1536)
    return {
        "eye": np.eye(128, dtype=f).astype(bf),
        "w_in": wiT.astype(bf),
        "w4": w4.astype(bf),
        "w_out": (np.ascontiguousarray(np.transpose(inputs["out_proj"], (0, 2, 1))) * 0.5).astype(bf),
        "w_xp": xp.astype(bf),
        "w_dt": np.ascontiguousarray(np.transpose(inputs["dt_w"], (0, 2, 1))).astype(bf),
        "cwd": diag.astype(bf),
        "cbh": (0.5 * inputs["conv_b"]).astype(f),
        "dtb": inputs["dt_b"].astype(f),
        "An": (-np.exp(inputs["A_log"])).astype(f),
        "dp": (0.5 * inputs["Dp"]).astype(f),
    }


def build_module_repeat(k):
    return build_module(False, repeat=k)


def kernel(**inputs):
    inputs = {k: np.asarray(v) for k, v in inputs.items()}
    nc = _get_nc(False)
    shared = prep_host(inputs)
    import ml_dtypes
    in_maps = []
    for s in range(BATCH):
        m = dict(shared)
        xt = np.ascontiguousarray(inputs["x"][s].T).astype(np.float32)
        m["xT"] = xt
        m["xTb"] = xt.astype(ml_dtypes.bfloat16)
        in_maps.append(m)
    res = run_bass_kernel_spmd(nc, in_maps, list(range(BATCH)))
    out = np.stack([res.results[s]["outT"].T for s in range(BATCH)])
    return np.ascontiguousarray(out.astype(np.float32))

